# revision 7
# baseline (speedup 1.0000x reference)
"""Trainium2 Bass kernel for nn_DLUPack (CARAFE-style dynamic upsampling).

Sharding: 8 cores = (batch n in [0,4)) x (output-row-parity s in {0,1});
core (n, s) computes low-res rows hh in [32s, 32s+32) -> all parity-s output rows.

Reference output mapping (its reshape scrambles positions):
  ref[n, c, 2y+i, 2x+j] = sum_k patches[c, hh, ww, k] * kern[hh, ww, k, u]
  with hh = 32s + 16jh + m:  row r = 8m + 2(ww//16) + s, col = 8*(ww%16) + 2u + jh.

Device pipeline per core (all post-conv tensors packed [128 = (jh, w)], FD halved
vs the [64, 2*FD] layout so DVE ops run in half the cycles):
  1. compressor 1x1 conv (PE, fp16) -> cx [64, 38, 66]
  2. offset+mask 3x3 convs (9 accumulated MMs) -> psum [57, .]: off ch 0-7, mask 32-56
     (mask channels host-permuted to k = kx*5 + ky order)
  3. exp in ACT evac; 20 PE-transposes of row-pairs (r, r+16) -> expT [128,20,25] f32
     softmax denom via free-dim reduce; expT4 = expT*recT bcast-u -> fp16 [128,20,100]
     partition-shifted copies expT4_{p1,m1} via SBUF-SBUF DMA (edges zeroed from zed)
  4. offset PE-transpose (row-pairs) -> deltT [128,16,8]; indicator bilinear W9 (DVE)
  5. kernc assembly [128, 16m, 25k, 4u]: 9 bcast-mult (1x) + 8 dense adds (2x)
  6. collate: 5 partition-shifted strided DMAs kernc -> data_all [128, 16, 100]
     (contaminated edge slots are skipped by the scatter idx tables = -1)
  7. per pair m: 2 local_scatter (GPSIMD) -> banded [128, 3x512 | 2x512]
  8. carafe: 5 accumulated MMs [128,128]x[128,512] per (pair, c-half) -> psum [128,512]
  9. ACT evac -> DMA out 4 contiguous output rows
"""
import sys
import numpy as np

sys.path.insert(0, '/opt/trn_rl_repo')

import ml_dtypes  # noqa: E402
from contextlib import ExitStack  # noqa: E402

import concourse.bass as bass  # noqa: E402
import concourse.tile as tile  # noqa: E402
from concourse import mybir, bacc  # noqa: E402
from concourse.bass_utils import run_bass_kernel_spmd  # noqa: E402

F32 = mybir.dt.float32
BF16 = mybir.dt.float16  # NOTE: fp16 (better mantissa), name kept for brevity
I16 = mybir.dt.int16
AF = mybir.ActivationFunctionType
OP = mybir.AluOpType

N, C, H, W = 4, 256, 64, 64


def _ap(base, off_elems, dims):
    return bass.AP(tensor=base.tensor, offset=base.offset + off_elems, ap=[list(d) for d in dims])


def build_scatter_tables():
    idx1 = -np.ones((128, 100), np.int16)
    idx2 = -np.ones((128, 100), np.int16)
    for p in range(128):
        jh, wpp = p // 64, p % 64
        for b in range(5):
            w = wpp + b - 2
            if not (0 <= w < 64):
                continue
            q, wl = w // 16, w % 16
            for ki in range(5):
                for u in range(4):
                    col = q * 128 + 8 * wl + 2 * u + jh
                    qidx = (b * 5 + ki) * 4 + u
                    if ki < 3:
                        idx1[p, qidx] = ki * 512 + col
                    else:
                        idx2[p, qidx] = (ki - 3) * 512 + col
    return idx1, idx2


def build_program():
    nc = bacc.Bacc(None, target_bir_lowering=False, debug=True)

    xwin = nc.declare_dram_parameter('xwin', [2, 128, 38 * 64], BF16, isOutput=False)
    xT2 = nc.declare_dram_parameter('xT2', [128, 20 * 256], BF16, isOutput=False)
    wc = nc.declare_dram_parameter('wc', [128, 2 * 64], BF16, isOutput=False)
    wk = nc.declare_dram_parameter('wk', [64, 9 * 57], BF16, isOutput=False)
    bco = nc.declare_dram_parameter('bco', [57, 1], F32, isOutput=False)
    bcomp = nc.declare_dram_parameter('bcomp', [64, 1], F32, isOutput=False)
    wvec = nc.declare_dram_parameter('wvec', [128, 1], F32, isOutput=False)
    w63 = nc.declare_dram_parameter('w63', [128, 1], F32, isOutput=False)
    hrow = nc.declare_dram_parameter('hrow', [128, 16], F32, isOutput=False)
    y63 = nc.declare_dram_parameter('y63', [128, 16], F32, isOutput=False)
    ident = nc.declare_dram_parameter('ident', [128, 128], F32, isOutput=False)
    idx1 = nc.declare_dram_parameter('idx1', [128, 100], I16, isOutput=False)
    idx2 = nc.declare_dram_parameter('idx2', [128, 100], I16, isOutput=False)
    zed = nc.declare_dram_parameter('zed', [2, 2000], BF16, isOutput=False)
    outp = nc.declare_dram_parameter('outp', [256, 64 * 128], F32, isOutput=True)

    with tile.TileContext(nc) as tc, ExitStack() as ctx:
        sing = ctx.enter_context(tc.tile_pool(name='sing', bufs=1))
        work = ctx.enter_context(tc.tile_pool(name='work', bufs=1))
        band = ctx.enter_context(tc.tile_pool(name='band', bufs=4))
        rowp = ctx.enter_context(tc.tile_pool(name='rowp', bufs=4))
        psum = ctx.enter_context(tc.psum_pool(name='ps', bufs=2))
        psc = ctx.enter_context(tc.psum_pool(name='psc', bufs=3))

        def load(shape, dtype, src, eng=None):
            t = sing.tile(shape, dtype, name=f'ld_{src.tensor.name if hasattr(src, "tensor") else id(src)}')
            (eng or nc.sync).dma_start(out=t[:], in_=src[:])
            return t

        # small params first on sync queue so compute can start ASAP;
        # ident early (warm-up matmuls need it)
        wc_sb = load([128, 2, 64], BF16, wc)
        bcomp_sb = load([64, 1], F32, bcomp)
        id_sb = load([128, 128], F32, ident)
        wk_sb = load([64, 9, 57], BF16, wk)
        bco_sb = load([57, 1], F32, bco)
        # xwin split across two queues; xT2 (only needed at carafe time) on a third
        xwin_sb = sing.tile([128, 2, 38 * 64], BF16)
        nc.sync.dma_start(out=xwin_sb[:, 0, :],
                          in_=_ap(xwin[:], 0, [[2432, 128], [1, 2432]]))
        nc.scalar.dma_start(out=xwin_sb[:, 1, :],
                            in_=_ap(xwin[:], 128 * 2432, [[2432, 128], [1, 2432]]))
        wvec_sb = load([128, 1], F32, wvec, eng=nc.gpsimd)
        w63_sb = load([128, 1], F32, w63, eng=nc.gpsimd)
        hrow_sb = load([128, 16], F32, hrow, eng=nc.gpsimd)
        y63_sb = load([128, 16], F32, y63, eng=nc.gpsimd)
        xT2_sb = load([128, 20 * 256], BF16, xT2, eng=nc.scalar)
        idx1_sb = load([128, 100], I16, idx1, eng=nc.scalar)
        idx2_sb = load([128, 100], I16, idx2, eng=nc.scalar)

        # PE warm-up: keep TensorE busy during input-DMA wait so HAM reaches 8/8
        pw = psc.tile([128, 512], F32, name='pcs_warm', tag='pcs')
        for _ in range(90):
            nc.tensor.matmul(pw[0:64, 0:64], id_sb[:, 0:64], id_sb[:, 0:64], start=True, stop=True)

        # ---- 1. compressor ----
        cx_sb = work.tile([64, 38, 66], BF16)
        nc.vector.memset(_ap(cx_sb[:], 0, [[38 * 66, 64], [66, 38], [1, 1]]), 0.0)
        nc.vector.memset(_ap(cx_sb[:], 65, [[38 * 66, 64], [66, 38], [1, 1]]), 0.0)
        for grp in range(5):
            g0 = grp * 8
            rows = min(8, 38 - g0)
            nn = rows * 64
            pcs = psum.tile([64, 512], F32)
            for cg in range(2):
                nc.tensor.matmul(pcs[:, :nn], wc_sb[:, cg, :],
                                 xwin_sb[:, cg, g0 * 64:g0 * 64 + nn],
                                 start=(cg == 0), stop=(cg == 1))
            nc.scalar.activation(
                out=_ap(cx_sb[:], g0 * 66 + 1, [[38 * 66, 64], [66, rows], [1, 64]]),
                in_=_ap(pcs[:], 0, [[512, 64], [64, rows], [1, 64]]),
                func=AF.Identity, bias=bcomp_sb[:], scale=1.0)

        # ---- 2. offset+mask convs ----
        # outputs stored pair-interleaved (i, jh, w) so transposes read one
        # contiguous [., 128] block; conv row r = 16jh + i covers h = r - 2
        expS = work.tile([25, 20, 2, 64], F32)
        offS = work.tile([8, 16, 2, 64], F32)
        for jh in range(2):
            for grp in range(4):
                i0 = grp * 5
                nn = 5 * 64
                pcs = psum.tile([57, 320], F32)
                for t in range(9):
                    dy, dx = t // 3, t % 3
                    rhs = _ap(cx_sb[:], (16 * jh + i0 + dy) * 66 + dx,
                              [[38 * 66, 64], [66, 5], [1, 64]])
                    nc.tensor.matmul(pcs[:, :nn], wk_sb[:, t, :], rhs,
                                     start=(t == 0), stop=(t == 8))
                nc.scalar.activation(
                    out=_ap(expS[:], (i0 * 2 + jh) * 64, [[20 * 128, 25], [128, 5], [1, 64]]),
                    in_=_ap(pcs[:], 32 * 320, [[320, 25], [64, 5], [1, 64]]),
                    func=AF.Exp, bias=bco_sb[32:57], scale=1.0)
                lo, hi = max(i0, 2), min(i0 + 5, 18)
                if lo < hi:
                    nc.vector.tensor_scalar(
                        out=_ap(offS[:], ((lo - 2) * 2 + jh) * 64,
                                [[16 * 128, 8], [128, hi - lo], [1, 64]]),
                        in0=_ap(pcs[:], (lo - i0) * 64, [[320, 8], [64, hi - lo], [1, 64]]),
                        scalar1=bco_sb[0:8], scalar2=None, op0=OP.add)

        # ---- 4. offset transpose (pairs (m, m+16) -> partitions (jh, w)); W9 ----
        deltT = work.tile([128, 16, 8], BF16)
        po_t = psc.tile([128, 512], F32, name='po_w', tag='pcs')
        for m in range(16):
            nc.tensor.transpose(po_t[:, m * 8:m * 8 + 8],
                                _ap(offS[:], m * 128, [[16 * 128, 8], [1, 128]]),
                                id_sb[0:8, 0:8])
        nc.scalar.activation(
            out=deltT[:],
            in_=_ap(po_t[:], 0, [[512, 128], [1, 128]]),
            func=AF.Copy, scale=1.0)

        def dview(chbase):
            return _ap(deltT[:], chbase, [[128, 128], [8, 16], [1, 4]])

        def wt(nm):
            return work.tile([128, 64], BF16, name=nm)

        t1, t2 = wt('t1'), wt('t2')
        gxc, x0r, wxt, omwx, x1r = wt('gxc'), wt('x0r'), wt('wxt'), wt('omwx'), wt('x1r')
        gyc, y0r, wyt, omwy, y1r = wt('gyc'), wt('y0r'), wt('wyt'), wt('omwy'), wt('y1r')
        ia, ib = wt('ia'), wt('ib')
        cwx = work.tile([128, 3, 64], BF16)
        rwy = work.tile([128, 3, 64], BF16)
        W9b = work.tile([128, 9 * 64], BF16)

        hrow_bc = _ap(hrow_sb[:], 0, [[16, 128], [1, 16], [0, 4]])
        y63_bc = _ap(y63_sb[:], 0, [[16, 128], [1, 16], [0, 4]])

        def r4(ap):
            return _ap(ap, 0, [[64, 128], [4, 16], [1, 4]])

        nc.vector.tensor_scalar(out=t1[:], in0=dview(0), scalar1=wvec_sb[:], scalar2=None, op0=OP.add)
        nc.vector.tensor_scalar(out=t2[:], in0=t1[:], scalar1=0.0, scalar2=63.0, op0=OP.max, op1=OP.min)
        nc.vector.tensor_scalar(out=gxc[:], in0=t2[:], scalar1=wvec_sb[:], scalar2=None, op0=OP.subtract)
        nc.vector.tensor_scalar(out=x0r[:], in0=gxc[:], scalar1=0.0, scalar2=-1.0, op0=OP.is_lt, op1=OP.mult)
        nc.vector.tensor_tensor(out=wxt[:], in0=gxc[:], in1=x0r[:], op=OP.subtract)
        nc.vector.tensor_scalar(out=omwx[:], in0=wxt[:], scalar1=-1.0, scalar2=1.0, op0=OP.mult, op1=OP.add)
        nc.vector.tensor_scalar(out=x1r[:], in0=x0r[:], scalar1=1.0, scalar2=w63_sb[:], op0=OP.add, op1=OP.min)

        nc.vector.tensor_tensor(out=r4(t1[:]), in0=dview(4), in1=hrow_bc, op=OP.add)
        nc.vector.tensor_scalar(out=t2[:], in0=t1[:], scalar1=0.0, scalar2=63.0, op0=OP.max, op1=OP.min)
        nc.vector.tensor_tensor(out=r4(gyc[:]), in0=r4(t2[:]), in1=hrow_bc, op=OP.subtract)
        nc.vector.tensor_scalar(out=y0r[:], in0=gyc[:], scalar1=0.0, scalar2=-1.0, op0=OP.is_lt, op1=OP.mult)
        nc.vector.tensor_tensor(out=wyt[:], in0=gyc[:], in1=y0r[:], op=OP.subtract)
        nc.vector.tensor_scalar(out=omwy[:], in0=wyt[:], scalar1=-1.0, scalar2=1.0, op0=OP.mult, op1=OP.add)
        nc.vector.tensor_scalar(out=t1[:], in0=y0r[:], scalar1=1.0, scalar2=None, op0=OP.add)
        nc.vector.tensor_tensor(out=r4(y1r[:]), in0=r4(t1[:]), in1=y63_bc, op=OP.min)

        for i, e in enumerate((-1.0, 0.0, 1.0)):
            nc.vector.tensor_scalar(out=ia[:], in0=x0r[:], scalar1=e, scalar2=None, op0=OP.is_equal)
            nc.vector.tensor_scalar(out=ib[:], in0=x1r[:], scalar1=e, scalar2=None, op0=OP.is_equal)
            nc.vector.tensor_tensor(out=ia[:], in0=ia[:], in1=omwx[:], op=OP.mult)
            nc.vector.tensor_tensor(out=ib[:], in0=ib[:], in1=wxt[:], op=OP.mult)
            nc.vector.tensor_tensor(out=cwx[:, i, :], in0=ia[:], in1=ib[:], op=OP.add)
            nc.vector.tensor_scalar(out=ia[:], in0=y0r[:], scalar1=e, scalar2=None, op0=OP.is_equal)
            nc.vector.tensor_scalar(out=ib[:], in0=y1r[:], scalar1=e, scalar2=None, op0=OP.is_equal)
            nc.vector.tensor_tensor(out=ia[:], in0=ia[:], in1=omwy[:], op=OP.mult)
            nc.vector.tensor_tensor(out=ib[:], in0=ib[:], in1=wyt[:], op=OP.mult)
            nc.vector.tensor_tensor(out=rwy[:, i, :], in0=ia[:], in1=ib[:], op=OP.add)
        for iy in range(3):
            for ix in range(3):
                nc.vector.tensor_tensor(
                    out=_ap(W9b[:], (iy * 3 + ix) * 64, [[9 * 64, 128], [1, 64]]),
                    in0=rwy[:, iy, :], in1=cwx[:, ix, :], op=OP.mult)

        # ---- 3. transpose exp (row-pairs (r, r+16)) -> expT [128, 20, 25]; softmax ----
        expT = work.tile([128, 20, 25], F32)
        pt = psc.tile([128, 512], F32, name='pt_exp', tag='pcs')
        for i in range(20):
            # exp rows h = 16jh + i - 2 for jh in {0,1} -> out parts (jh, w)
            nc.tensor.transpose(pt[:, i * 25:i * 25 + 25],
                                _ap(expS[:], i * 128, [[20 * 128, 25], [1, 128]]),
                                id_sb[0:25, 0:25])
        nc.scalar.activation(
            out=expT[:],
            in_=_ap(pt[:], 0, [[512, 128], [1, 500]]),
            func=AF.Copy, scale=1.0)
        sumT = work.tile([128, 20], F32)
        nc.vector.tensor_reduce(out=sumT[:], in_=expT[:], axis=mybir.AxisListType.X, op=OP.add)
        recT = work.tile([128, 20], F32)
        nc.vector.reciprocal(out=recT[:], in_=sumT[:])

        # expT4 = expT * recT, broadcast over u -> fp16 [128, 20, 25, 4]
        expT4 = work.tile([128, 20, 25, 4], BF16)
        nc.vector.tensor_tensor(
            out=expT4[:],
            in0=_ap(expT[:], 0, [[500, 128], [25, 20], [1, 25], [0, 4]]),
            in1=_ap(recT[:], 0, [[20, 128], [1, 20], [0, 25], [0, 4]]), op=OP.mult)
        # partition-shifted variants (w +- 1); cross-jh contamination rows get zero
        # W9 weight by construction; edge partitions filled with zeros from DRAM
        expT4_p1 = work.tile([128, 20, 25, 4], BF16)   # [p] = expT4[p+1]
        expT4_m1 = work.tile([128, 20, 25, 4], BF16)   # [p] = expT4[p-1]
        nc.sync.dma_start(out=_ap(expT4_p1[:], 0, [[2000, 127], [1, 2000]]),
                          in_=_ap(expT4[:], 2000, [[2000, 127], [1, 2000]]))
        nc.scalar.dma_start(out=_ap(expT4_m1[:], 2000, [[2000, 127], [1, 2000]]),
                            in_=_ap(expT4[:], 0, [[2000, 127], [1, 2000]]))
        nc.sync.dma_start(out=_ap(expT4_p1[:], 127 * 2000, [[2000, 1], [1, 2000]]),
                          in_=_ap(zed[:], 0, [[2000, 1], [1, 2000]]))
        nc.scalar.dma_start(out=_ap(expT4_m1[:], 0, [[2000, 1], [1, 2000]]),
                            in_=_ap(zed[:], 0, [[2000, 1], [1, 2000]]))

        # ---- 5-9 software-pipelined by m-groups ----
        kernc = work.tile([128, 16, 25, 4], BF16)   # k = kx*5 + ky (host-permuted)
        tmp = work.tile([128, 400], BF16)
        data_all = work.tile([128, 16, 100], BF16)
        exp_by_ex = {-1: expT4_m1, 0: expT4, 1: expT4_p1}
        NG = 4
        GM = 16 // NG

        def emit_asm(G):
            kv = _ap(kernc[:], GM * G * 100, [[1600, 128], [100, GM], [1, 100]])
            tv = _ap(tmp[:], 0, [[400, 128], [1, 400]])
            first = True
            for iy, ey in enumerate((-1, 0, 1)):
                for ix, ex in enumerate((-1, 0, 1)):
                    mv = _ap(exp_by_ex[ex][:], (2 + ey + GM * G) * 100,
                             [[2000, 128], [100, GM], [1, 100]])
                    wv = _ap(W9b[:], (iy * 3 + ix) * 64 + GM * G * 4,
                             [[9 * 64, 128], [4, GM], [0, 25], [1, 4]])
                    if first:
                        kvb = _ap(kernc[:], GM * G * 100,
                                  [[1600, 128], [100, GM], [4, 25], [1, 4]])
                        mvb = _ap(exp_by_ex[ex][:], (2 + ey + GM * G) * 100,
                                  [[2000, 128], [100, GM], [4, 25], [1, 4]])
                        nc.vector.tensor_tensor(out=kvb, in0=wv, in1=mvb, op=OP.mult)
                        first = False
                    else:
                        tvb = _ap(tmp[:], 0, [[400, 128], [100, GM], [4, 25], [1, 4]])
                        mvb = _ap(exp_by_ex[ex][:], (2 + ey + GM * G) * 100,
                                  [[2000, 128], [100, GM], [4, 25], [1, 4]])
                        nc.vector.tensor_tensor(out=tvb, in0=wv, in1=mvb, op=OP.mult)
                        nc.vector.tensor_tensor(out=kv, in0=kv, in1=tv, op=OP.add)

        def emit_collate(G):
            # data_all[p, m, b*20 + (ki,u)] = kernc[p + (b-2), m, (4-b)*20 + (ki,u)]
            # out-of-range/contaminated slots are skipped by idx tables (-1)
            for b in range(5):
                d = b - 2
                cnt = 128 - abs(d)
                in_off = max(0, d) * 1600 + (4 - b) * 20 + GM * G * 100
                out_off = max(0, -d) * 1600 + b * 20 + GM * G * 100
                nc.sync.dma_start(
                    out=_ap(data_all[:], out_off, [[1600, cnt], [100, GM], [1, 20]]),
                    in_=_ap(kernc[:], in_off, [[1600, cnt], [100, GM], [1, 20]]))

        def emit_pairs(G):
            for m in range(GM * G, GM * G + GM):
                banded1 = band.tile([128, 1536], BF16, name=f'band1_{m}', tag='band1')
                banded2 = band.tile([128, 1024], BF16, name=f'band2_{m}', tag='band2')
                nc.gpsimd.local_scatter(out_ap=banded1[:], data_ap=data_all[:, m, :],
                                        idxs_ap=idx1_sb[:], channels=128, num_elems=1536, num_idxs=100)
                nc.gpsimd.local_scatter(out_ap=banded2[:], data_ap=data_all[:, m, :],
                                        idxs_ap=idx2_sb[:], channels=128, num_elems=1024, num_idxs=100)
                for ch in range(2):
                    pcs = psc.tile([128, 512], F32, name=f'pcs_{m}_{ch}', tag='pcs')
                    for ki in range(5):
                        lhsT = _ap(xT2_sb[:], (m + ki) * 256 + ch * 128, [[20 * 256, 128], [1, 128]])
                        rhs = banded1[:, ki * 512:ki * 512 + 512] if ki < 3 \
                            else banded2[:, (ki - 3) * 512:(ki - 3) * 512 + 512]
                        nc.tensor.matmul(pcs[:], lhsT, rhs, start=(ki == 0), stop=(ki == 4))
                    rb = rowp.tile([128, 512], F32, name=f'rb_{m}_{ch}', tag='rb')
                    nc.scalar.activation(out=rb[:], in_=pcs[:], func=AF.Copy, scale=1.0)
                    nc.scalar.dma_start(
                        out=_ap(outp[:], ch * 128 * 8192 + 4 * m * 128,
                                [[8192, 128], [128, 4], [1, 128]]),
                        in_=rb[:])

        emit_asm(0)
        emit_collate(0)
        for G in range(NG):
            if G + 1 < NG:
                emit_asm(G + 1)
                emit_collate(G + 1)
            emit_pairs(G)
    nc.finalize()
    return nc


_PROGRAM = None
_SCAT = build_scatter_tables()


def _get_program():
    global _PROGRAM
    if _PROGRAM is None:
        _PROGRAM = build_program()
    return _PROGRAM


def _prep_core_inputs(inputs, n, s):
    bf = np.float16
    x = np.asarray(inputs['x'][n], np.float32)
    h0 = 32 * s
    xw = np.zeros((C, 38, W), np.float32)
    for i, g in enumerate(range(h0 - 3, h0 + 35)):
        if 0 <= g < H:
            xw[:, i] = x[:, g]
    xwin = np.ascontiguousarray(xw.reshape(2, 128, 38 * 64)).astype(bf)
    xT2 = np.zeros((128, 20, C), np.float32)
    for jh in range(2):
        base = h0 + 16 * jh - 2
        for i in range(20):
            g = base + i
            if 0 <= g < H:
                xT2[64 * jh:64 * jh + 64, i] = x[:, g].T
    xT2 = np.ascontiguousarray(xT2.reshape(128, 20 * 256)).astype(bf)
    w_comp = np.asarray(inputs['w_comp'], np.float32)[:, :, 0, 0]
    wc = np.zeros((2, 128, 64), np.float32)
    for cg in range(2):
        wc[cg] = w_comp[:, cg * 128:(cg + 1) * 128].T
    wc = np.ascontiguousarray(wc.transpose(1, 0, 2).reshape(128, 2 * 64)).astype(bf)
    w_ker = np.asarray(inputs['w_ker'], np.float32)
    w_off = np.asarray(inputs['w_off'], np.float32)
    # mask channels permuted to k = kx*5 + ky so the collate DMA reads a
    # contiguous 20-elem (ki, u) slice per horizontal tap kx
    kperm = np.array([ky * 5 + kx for kx in range(5) for ky in range(5)])
    wk = np.zeros((9, 64, 57), np.float32)
    for t in range(9):
        wk[t, :, 0:8] = w_off[:, :, t // 3, t % 3].T
        wk[t, :, 32:57] = w_ker[kperm, :, t // 3, t % 3].T
    wk = np.ascontiguousarray(wk.transpose(1, 0, 2).reshape(64, 9 * 57)).astype(bf)
    bcov = np.zeros((57, 1), np.float32)
    bcov[0:8, 0] = np.asarray(inputs['b_off'], np.float32)
    bcov[32:57, 0] = np.asarray(inputs['b_ker'], np.float32)[kperm]
    idx1, idx2 = _SCAT
    wv = np.tile(np.arange(64, dtype=np.float32), 2).reshape(128, 1)
    hr = (h0 + 16 * (np.arange(128)[:, None] // 64)
          + np.arange(16, dtype=np.float32)[None, :]).astype(np.float32)
    return {
        'xwin': xwin, 'xT2': xT2, 'wc': wc, 'wk': wk, 'bco': bcov,
        'bcomp': np.asarray(inputs['b_comp'], np.float32).reshape(64, 1),
        'wvec': wv,
        'w63': (63.0 - wv).astype(np.float32),
        'hrow': np.ascontiguousarray(hr),
        'y63': np.ascontiguousarray(63.0 - hr),
        'ident': np.eye(128, dtype=np.float32),
        'idx1': idx1, 'idx2': idx2,
        'zed': np.zeros((2, 2000), np.float16),
    }


def kernel(**inputs):
    nc = _get_program()
    core_ids = list(range(8))
    in_maps = [_prep_core_inputs(inputs, cid // 2, cid % 2) for cid in core_ids]
    res = run_bass_kernel_spmd(nc, in_maps, core_ids)
    out = np.zeros((N, C, 128, 128), np.float32)
    for cid in core_ids:
        n, s = cid // 2, cid % 2
        op = np.asarray(res.results[cid]['outp']).reshape(256, 64, 128)
        out[n, :, s::2] = op
    return out


if __name__ == '__main__':
    d = np.load('/root/problem/ref_io.npz')
    inp = {k: d[k] for k in ('x', 'w_comp', 'b_comp', 'w_ker', 'b_ker', 'w_off', 'b_off')}
    out = kernel(**inp)
    ref = d['out']
    err = np.abs(out - ref).max()
    print('max abs err:', err, 'rel:', err / np.abs(ref).max())


# revision 16
# speedup vs baseline: 1.4260x; 1.4260x over previous
"""Trainium2 Bass kernel for nn_DLUPack (CARAFE-style dynamic upsampling).

Sharding: 8 cores = (batch n in [0,4)) x (output-row-parity s in {0,1});
core (n, s) computes low-res rows hh in [32s, 32s+32) -> all parity-s output rows.

Reference output mapping (its reshape scrambles positions):
  ref[n, c, 2y+i, 2x+j] = sum_k patches[c, hh, ww, k] * kern[hh, ww, k, u]
  with hh = 32s + 16jh + m:  row r = 8m + 2(ww//16) + s, col = 8*(ww%16) + 2u + jh.

Device pipeline per core (all post-conv tensors packed [128 = (jh, w)], FD halved
vs the [64, 2*FD] layout so DVE ops run in half the cycles):
  1. compressor 1x1 conv (PE, fp16) -> cx [64, 38, 66]
  2. offset+mask 3x3 convs (9 accumulated MMs) -> psum [57, .]: off ch 0-7, mask 32-56
     (mask channels host-permuted to k = kx*5 + ky order)
  3. exp in ACT evac; 20 PE-transposes of row-pairs (r, r+16) -> expT [128,20,25] f32
     softmax denom via free-dim reduce; expT4 = expT*recT bcast-u -> fp16 [128,20,100]
     partition-shifted copies expT4_{p1,m1} via SBUF-SBUF DMA (edges zeroed from zed)
  4. offset PE-transpose (row-pairs) -> deltT [128,16,8]; indicator bilinear W9 (DVE)
  5. kernc assembly [128, 16m, 25k, 4u]: 9 bcast-mult (1x) + 8 dense adds (2x)
  6. collate: 5 partition-shifted strided DMAs kernc -> data_all [128, 16, 100]
     (contaminated edge slots are skipped by the scatter idx tables = -1)
  7. per pair m: 2 local_scatter (GPSIMD) -> banded [128, 3x512 | 2x512]
  8. carafe: 5 accumulated MMs [128,128]x[128,512] per (pair, c-half) -> psum [128,512]
  9. ACT evac -> DMA out 4 contiguous output rows
"""
import sys
import numpy as np

sys.path.insert(0, '/opt/trn_rl_repo')

import ml_dtypes  # noqa: E402
from contextlib import ExitStack  # noqa: E402

import concourse.bass as bass  # noqa: E402
import concourse.tile as tile  # noqa: E402
from concourse import mybir, bacc  # noqa: E402
from concourse.bass_utils import run_bass_kernel_spmd  # noqa: E402

F32 = mybir.dt.float32
BF16 = mybir.dt.float16  # NOTE: fp16 (better mantissa), name kept for brevity
I16 = mybir.dt.int16
AF = mybir.ActivationFunctionType
OP = mybir.AluOpType

N, C, H, W = 4, 256, 64, 64


def _ap(base, off_elems, dims):
    return bass.AP(tensor=base.tensor, offset=base.offset + off_elems, ap=[list(d) for d in dims])


def build_scatter_tables():
    idx1 = -np.ones((128, 100), np.int16)
    idx2 = -np.ones((128, 100), np.int16)
    for p in range(128):
        jh, wpp = p // 64, p % 64
        for b in range(5):
            w = wpp + b - 2
            if not (0 <= w < 64):
                continue
            q, wl = w // 16, w % 16
            for ki in range(5):
                for u in range(4):
                    col = q * 128 + 8 * wl + 2 * u + jh
                    qidx = (b * 5 + ki) * 4 + u
                    if ki < 3:
                        idx1[p, qidx] = ki * 512 + col
                    else:
                        idx2[p, qidx] = (ki - 3) * 512 + col
    return idx1, idx2


def build_program():
    nc = bacc.Bacc(None, target_bir_lowering=False, debug=True)

    xwin = nc.declare_dram_parameter('xwin', [2, 128, 38 * 64], BF16, isOutput=False)
    xT2 = nc.declare_dram_parameter('xT2', [128, 20 * 256], BF16, isOutput=False)
    wc = nc.declare_dram_parameter('wc', [128, 2 * 64], BF16, isOutput=False)
    wk = nc.declare_dram_parameter('wk', [64, 9 * 57], BF16, isOutput=False)
    bco = nc.declare_dram_parameter('bco', [57, 1], F32, isOutput=False)
    bcomp = nc.declare_dram_parameter('bcomp', [64, 1], F32, isOutput=False)
    wvec = nc.declare_dram_parameter('wvec', [128, 1], F32, isOutput=False)
    w63 = nc.declare_dram_parameter('w63', [128, 1], F32, isOutput=False)
    hrow = nc.declare_dram_parameter('hrow', [128, 16], F32, isOutput=False)
    y63 = nc.declare_dram_parameter('y63', [128, 16], F32, isOutput=False)
    ident = nc.declare_dram_parameter('ident', [128, 128], F32, isOutput=False)
    shmat = nc.declare_dram_parameter('shmat', [128, 5 * 128], BF16, isOutput=False)
    idx1 = nc.declare_dram_parameter('idx1', [128, 100], I16, isOutput=False)
    idx2 = nc.declare_dram_parameter('idx2', [128, 100], I16, isOutput=False)
    outp = nc.declare_dram_parameter('outp', [256, 64 * 128], F32, isOutput=True)

    with tile.TileContext(nc) as tc, ExitStack() as ctx:
        sing = ctx.enter_context(tc.tile_pool(name='sing', bufs=1))
        work = ctx.enter_context(tc.tile_pool(name='work', bufs=1))
        band = ctx.enter_context(tc.tile_pool(name='band', bufs=4))
        rowp = ctx.enter_context(tc.tile_pool(name='rowp', bufs=4))
        psum = ctx.enter_context(tc.psum_pool(name='ps', bufs=2))
        psc = ctx.enter_context(tc.psum_pool(name='psc', bufs=3))

        def load(shape, dtype, src, eng=None):
            t = sing.tile(shape, dtype, name=f'ld_{src.tensor.name if hasattr(src, "tensor") else id(src)}')
            (eng or nc.sync).dma_start(out=t[:], in_=src[:])
            return t

        # small params first on sync queue so compute can start ASAP;
        # ident early (warm-up matmuls need it)
        wc_sb = load([128, 2, 64], BF16, wc)
        bcomp_sb = load([64, 1], F32, bcomp)
        id_sb = load([128, 128], F32, ident)
        wk_sb = load([64, 9, 57], BF16, wk)
        bco_sb = load([57, 1], F32, bco)
        # xwin split across two queues; xT2 (only needed at carafe time) on a third
        xwin_sb = sing.tile([128, 2, 38 * 64], BF16)
        nc.sync.dma_start(out=xwin_sb[:, 0, :],
                          in_=_ap(xwin[:], 0, [[2432, 128], [1, 2432]]))
        nc.scalar.dma_start(out=xwin_sb[:, 1, :],
                            in_=_ap(xwin[:], 128 * 2432, [[2432, 128], [1, 2432]]))
        wvec_sb = load([128, 1], F32, wvec, eng=nc.gpsimd)
        w63_sb = load([128, 1], F32, w63, eng=nc.gpsimd)
        hrow_sb = load([128, 16], F32, hrow, eng=nc.gpsimd)
        y63_sb = load([128, 16], F32, y63, eng=nc.gpsimd)
        sh_sb = load([128, 5, 128], BF16, shmat, eng=nc.gpsimd)
        xT2_sb = load([128, 20 * 256], BF16, xT2, eng=nc.scalar)
        idx1_sb = load([128, 100], I16, idx1, eng=nc.scalar)
        idx2_sb = load([128, 100], I16, idx2, eng=nc.scalar)

        # PE warm-up: keep TensorE busy during input-DMA wait so HAM reaches 8/8
        pw = psc.tile([128, 512], F32, name='pcs_warm', tag='pcs')
        for _ in range(90):
            nc.tensor.matmul(pw[0:64, 0:64], id_sb[:, 0:64], id_sb[:, 0:64], start=True, stop=True)

        # ---- 1. compressor ----
        cx_sb = work.tile([64, 38, 66], BF16)
        nc.vector.memset(_ap(cx_sb[:], 0, [[38 * 66, 64], [66, 38], [1, 1]]), 0.0)
        nc.vector.memset(_ap(cx_sb[:], 65, [[38 * 66, 64], [66, 38], [1, 1]]), 0.0)
        for grp in range(5):
            g0 = grp * 8
            rows = min(8, 38 - g0)
            nn = rows * 64
            pcs = psum.tile([64, 512], F32)
            for cg in range(2):
                nc.tensor.matmul(pcs[:, :nn], wc_sb[:, cg, :],
                                 xwin_sb[:, cg, g0 * 64:g0 * 64 + nn],
                                 start=(cg == 0), stop=(cg == 1))
            nc.scalar.activation(
                out=_ap(cx_sb[:], g0 * 66 + 1, [[38 * 66, 64], [66, rows], [1, 64]]),
                in_=_ap(pcs[:], 0, [[512, 64], [64, rows], [1, 64]]),
                func=AF.Identity, bias=bcomp_sb[:], scale=1.0)

        # ---- 2. offset+mask convs ----
        # outputs stored pair-interleaved (i, jh, w) so transposes read one
        # contiguous [., 128] block; conv row r = 16jh + i covers h = r - 2
        expS = work.tile([25, 20, 2, 64], F32)
        offS = work.tile([8, 16, 2, 64], F32)
        for jh in range(2):
            for grp in range(4):
                i0 = grp * 5
                nn = 5 * 64
                pcs = psum.tile([57, 320], F32)
                for t in range(9):
                    dy, dx = t // 3, t % 3
                    rhs = _ap(cx_sb[:], (16 * jh + i0 + dy) * 66 + dx,
                              [[38 * 66, 64], [66, 5], [1, 64]])
                    nc.tensor.matmul(pcs[:, :nn], wk_sb[:, t, :], rhs,
                                     start=(t == 0), stop=(t == 8))
                nc.scalar.activation(
                    out=_ap(expS[:], (i0 * 2 + jh) * 64, [[20 * 128, 25], [128, 5], [1, 64]]),
                    in_=_ap(pcs[:], 32 * 320, [[320, 25], [64, 5], [1, 64]]),
                    func=AF.Exp, bias=bco_sb[32:57], scale=1.0)
                lo, hi = max(i0, 2), min(i0 + 5, 18)
                if lo < hi:
                    nc.vector.tensor_scalar(
                        out=_ap(offS[:], ((lo - 2) * 2 + jh) * 64,
                                [[16 * 128, 8], [128, hi - lo], [1, 64]]),
                        in0=_ap(pcs[:], (lo - i0) * 64, [[320, 8], [64, hi - lo], [1, 64]]),
                        scalar1=bco_sb[0:8], scalar2=None, op0=OP.add)

        # ---- 4. offset transpose (pairs (m, m+16) -> partitions (jh, w)); W9 ----
        deltT = work.tile([128, 16, 8], BF16)
        po_t = psc.tile([128, 512], F32, name='po_w', tag='pcs')
        for m in range(16):
            nc.tensor.transpose(po_t[:, m * 8:m * 8 + 8],
                                _ap(offS[:], m * 128, [[16 * 128, 8], [1, 128]]),
                                id_sb[0:8, 0:8])
        nc.scalar.activation(
            out=deltT[:],
            in_=_ap(po_t[:], 0, [[512, 128], [1, 128]]),
            func=AF.Copy, scale=1.0)

        def dview(chbase):
            return _ap(deltT[:], chbase, [[128, 128], [8, 16], [1, 4]])

        def wt(nm):
            return work.tile([128, 64], BF16, name=nm)

        t1, t2 = wt('t1'), wt('t2')
        gxc, x0r, wxt, omwx, x1r = wt('gxc'), wt('x0r'), wt('wxt'), wt('omwx'), wt('x1r')
        gyc, y0r, wyt, omwy, y1r = wt('gyc'), wt('y0r'), wt('wyt'), wt('omwy'), wt('y1r')
        ia, ib = wt('ia'), wt('ib')
        cwx = work.tile([128, 3, 64], BF16)
        rwy = work.tile([128, 3, 64], BF16)
        # W9 broadcast over ki: [t, m(16), ki(5), u(4)] so asm APs stay 3-free-dim
        W9b = work.tile([128, 9 * 320], BF16)

        hrow_bc = _ap(hrow_sb[:], 0, [[16, 128], [1, 16], [0, 4]])
        y63_bc = _ap(y63_sb[:], 0, [[16, 128], [1, 16], [0, 4]])

        def r4(ap):
            return _ap(ap, 0, [[64, 128], [4, 16], [1, 4]])

        nc.vector.tensor_scalar(out=t1[:], in0=dview(0), scalar1=wvec_sb[:], scalar2=None, op0=OP.add)
        nc.vector.tensor_scalar(out=t2[:], in0=t1[:], scalar1=0.0, scalar2=63.0, op0=OP.max, op1=OP.min)
        nc.vector.tensor_scalar(out=gxc[:], in0=t2[:], scalar1=wvec_sb[:], scalar2=None, op0=OP.subtract)
        nc.vector.tensor_scalar(out=x0r[:], in0=gxc[:], scalar1=0.0, scalar2=-1.0, op0=OP.is_lt, op1=OP.mult)
        nc.vector.tensor_tensor(out=wxt[:], in0=gxc[:], in1=x0r[:], op=OP.subtract)
        nc.vector.tensor_scalar(out=omwx[:], in0=wxt[:], scalar1=-1.0, scalar2=1.0, op0=OP.mult, op1=OP.add)
        nc.vector.tensor_scalar(out=x1r[:], in0=x0r[:], scalar1=1.0, scalar2=w63_sb[:], op0=OP.add, op1=OP.min)

        nc.vector.tensor_tensor(out=r4(t1[:]), in0=dview(4), in1=hrow_bc, op=OP.add)
        nc.vector.tensor_scalar(out=t2[:], in0=t1[:], scalar1=0.0, scalar2=63.0, op0=OP.max, op1=OP.min)
        nc.vector.tensor_tensor(out=r4(gyc[:]), in0=r4(t2[:]), in1=hrow_bc, op=OP.subtract)
        nc.vector.tensor_scalar(out=y0r[:], in0=gyc[:], scalar1=0.0, scalar2=-1.0, op0=OP.is_lt, op1=OP.mult)
        nc.vector.tensor_tensor(out=wyt[:], in0=gyc[:], in1=y0r[:], op=OP.subtract)
        nc.vector.tensor_scalar(out=omwy[:], in0=wyt[:], scalar1=-1.0, scalar2=1.0, op0=OP.mult, op1=OP.add)
        nc.vector.tensor_scalar(out=t1[:], in0=y0r[:], scalar1=1.0, scalar2=None, op0=OP.add)
        nc.vector.tensor_tensor(out=r4(y1r[:]), in0=r4(t1[:]), in1=y63_bc, op=OP.min)

        for i, e in enumerate((-1.0, 0.0, 1.0)):
            nc.vector.tensor_scalar(out=ia[:], in0=x0r[:], scalar1=e, scalar2=None, op0=OP.is_equal)
            nc.vector.tensor_scalar(out=ib[:], in0=x1r[:], scalar1=e, scalar2=None, op0=OP.is_equal)
            nc.vector.tensor_tensor(out=ia[:], in0=ia[:], in1=omwx[:], op=OP.mult)
            nc.vector.tensor_tensor(out=ib[:], in0=ib[:], in1=wxt[:], op=OP.mult)
            nc.vector.tensor_tensor(out=cwx[:, i, :], in0=ia[:], in1=ib[:], op=OP.add)
            nc.vector.tensor_scalar(out=ia[:], in0=y0r[:], scalar1=e, scalar2=None, op0=OP.is_equal)
            nc.vector.tensor_scalar(out=ib[:], in0=y1r[:], scalar1=e, scalar2=None, op0=OP.is_equal)
            nc.vector.tensor_tensor(out=ia[:], in0=ia[:], in1=omwy[:], op=OP.mult)
            nc.vector.tensor_tensor(out=ib[:], in0=ib[:], in1=wyt[:], op=OP.mult)
            nc.vector.tensor_tensor(out=rwy[:, i, :], in0=ia[:], in1=ib[:], op=OP.add)
        for iy in range(3):
            for ix in range(3):
                nc.vector.tensor_tensor(
                    out=_ap(W9b[:], (iy * 3 + ix) * 320,
                            [[9 * 320, 128], [20, 16], [4, 5], [1, 4]]),
                    in0=_ap(rwy[:], iy * 64, [[3 * 64, 128], [4, 16], [0, 5], [1, 4]]),
                    in1=_ap(cwx[:], ix * 64, [[3 * 64, 128], [4, 16], [0, 5], [1, 4]]),
                    op=OP.mult)

        # ---- 3. transpose exp (row-pairs (r, r+16)) -> expT [128, 20, 25]; softmax ----
        expT = work.tile([128, 20, 25], F32)
        pt = psc.tile([128, 512], F32, name='pt_exp', tag='pcs')
        for i in range(20):
            # exp rows h = 16jh + i - 2 for jh in {0,1} -> out parts (jh, w)
            nc.tensor.transpose(pt[:, i * 25:i * 25 + 25],
                                _ap(expS[:], i * 128, [[20 * 128, 25], [1, 128]]),
                                id_sb[0:25, 0:25])
        nc.scalar.activation(
            out=expT[:],
            in_=_ap(pt[:], 0, [[512, 128], [1, 500]]),
            func=AF.Copy, scale=1.0)
        sumT = work.tile([128, 20], F32)
        nc.vector.tensor_reduce(out=sumT[:], in_=expT[:], axis=mybir.AxisListType.X, op=OP.add)
        recT = work.tile([128, 20], F32)
        nc.vector.reciprocal(out=recT[:], in_=sumT[:])

        # expT4 = expT * recT, broadcast over u -> fp16 [128, 20, 25, 4]
        expT4 = work.tile([128, 20, 25, 4], BF16)
        nc.vector.tensor_tensor(
            out=expT4[:],
            in0=_ap(expT[:], 0, [[500, 128], [25, 20], [1, 25], [0, 4]]),
            in1=_ap(recT[:], 0, [[20, 128], [1, 20], [0, 25], [0, 4]]), op=OP.mult)
        # partition-shifted variants (w +- 1) via PE shift-matrix matmuls; the
        # shift matrices zero the out-of-block edges (SBUF-SBUF DMA shifts are
        # single-engine ~13GB/s -- far too slow)
        expT4_p1 = work.tile([128, 20, 25, 4], BF16)   # [p] = expT4[p+1]
        expT4_m1 = work.tile([128, 20, 25, 4], BF16)   # [p] = expT4[p-1]
        for dst, sidx in ((expT4_p1, 3), (expT4_m1, 1)):
            for ck in range(4):
                psh = psc.tile([128, 512], F32, name=f'psh_{sidx}_{ck}', tag='pcs')
                nc.tensor.matmul(psh[:, 0:500], sh_sb[:, sidx, :],
                                 _ap(expT4[:], ck * 500, [[2000, 128], [1, 500]]),
                                 start=True, stop=True)
                nc.scalar.activation(
                    out=_ap(dst[:], ck * 500, [[2000, 128], [1, 500]]),
                    in_=_ap(psh[:], 0, [[512, 128], [1, 500]]),
                    func=AF.Copy, scale=1.0)

        # ---- 5-9 software-pipelined by m-groups ----
        # kernc layout [p, kx(5), m(16), ki(5), u(4)] so per-(kx, m-group) slices
        # are contiguous for the collate shift-matmul rhs
        kernc = work.tile([128, 5, 16, 5, 4], BF16)
        tmp = work.tile([128, 400], BF16)
        data_all = work.tile([128, 16, 100], BF16)
        exp_by_ex = {-1: expT4_m1, 0: expT4, 1: expT4_p1}
        NG = 4
        GM = 16 // NG

        def emit_asm(G):
            kv = _ap(kernc[:], GM * G * 20, [[1600, 128], [320, 5], [20, GM], [1, 20]])
            tv = _ap(tmp[:], 0, [[400, 128], [80, 5], [20, GM], [1, 20]])
            first = True
            for iy, ey in enumerate((-1, 0, 1)):
                for ix, ex in enumerate((-1, 0, 1)):
                    # expT4 [row(20), kx(5), ki(5), u(4)] -> read as [kx, m, kiu]
                    mv = _ap(exp_by_ex[ex][:], (2 + ey + GM * G) * 100,
                             [[2000, 128], [20, 5], [100, GM], [1, 20]])
                    wv = _ap(W9b[:], (iy * 3 + ix) * 320 + GM * G * 20,
                             [[9 * 320, 128], [0, 5], [20, GM], [1, 20]])
                    if first:
                        nc.vector.tensor_tensor(out=kv, in0=wv, in1=mv, op=OP.mult)
                        first = False
                    else:
                        nc.vector.tensor_tensor(out=tv, in0=wv, in1=mv, op=OP.mult)
                        nc.vector.tensor_tensor(out=kv, in0=kv, in1=tv, op=OP.add)

        def emit_collate(G):
            # data_all[po, m, b*20+(ki,u)] = kernc[po+(b-2), kx=4-b, m, ki, u]
            # via PE shift matmuls (zero-padded at block edges by the matrices)
            pda = psc.tile([128, 512], F32, name=f'pda_{G}', tag='pcs')
            for b in range(5):
                rhs = _ap(kernc[:], (4 - b) * 320 + GM * G * 20, [[1600, 128], [1, 80]])
                nc.tensor.matmul(pda[:, b * 80:b * 80 + 80], sh_sb[:, b, :], rhs,
                                 start=True, stop=True)
            nc.scalar.activation(
                out=_ap(data_all[:], GM * G * 100, [[1600, 128], [100, GM], [20, 5], [1, 20]]),
                in_=_ap(pda[:], 0, [[512, 128], [20, GM], [80, 5], [1, 20]]),
                func=AF.Copy, scale=1.0)

        def emit_pairs(G):
            for m in range(GM * G, GM * G + GM):
                banded1 = band.tile([128, 1536], BF16, name=f'band1_{m}', tag='band1')
                banded2 = band.tile([128, 1024], BF16, name=f'band2_{m}', tag='band2')
                nc.gpsimd.local_scatter(out_ap=banded1[:], data_ap=data_all[:, m, :],
                                        idxs_ap=idx1_sb[:], channels=128, num_elems=1536, num_idxs=100)
                nc.gpsimd.local_scatter(out_ap=banded2[:], data_ap=data_all[:, m, :],
                                        idxs_ap=idx2_sb[:], channels=128, num_elems=1024, num_idxs=100)
                for ch in range(2):
                    pcs = psc.tile([128, 512], F32, name=f'pcs_{m}_{ch}', tag='pcs')
                    for ki in range(5):
                        lhsT = _ap(xT2_sb[:], (m + ki) * 256 + ch * 128, [[20 * 256, 128], [1, 128]])
                        rhs = banded1[:, ki * 512:ki * 512 + 512] if ki < 3 \
                            else banded2[:, (ki - 3) * 512:(ki - 3) * 512 + 512]
                        nc.tensor.matmul(pcs[:], lhsT, rhs, start=(ki == 0), stop=(ki == 4))
                    rb = rowp.tile([128, 512], F32, name=f'rb_{m}_{ch}', tag='rb')
                    nc.scalar.activation(out=rb[:], in_=pcs[:], func=AF.Copy, scale=1.0)
                    nc.sync.dma_start(
                        out=_ap(outp[:], ch * 128 * 8192 + 4 * m * 128,
                                [[8192, 128], [128, 4], [1, 128]]),
                        in_=rb[:])

        emit_asm(0)
        emit_collate(0)
        for G in range(NG):
            if G + 1 < NG:
                emit_asm(G + 1)
                emit_collate(G + 1)
            emit_pairs(G)
    nc.finalize()
    return nc


_PROGRAM = None
_SCAT = build_scatter_tables()


def _get_program():
    global _PROGRAM
    if _PROGRAM is None:
        _PROGRAM = build_program()
    return _PROGRAM


def _prep_core_inputs(inputs, n, s):
    bf = np.float16
    x = np.asarray(inputs['x'][n], np.float32)
    h0 = 32 * s
    xw = np.zeros((C, 38, W), np.float32)
    for i, g in enumerate(range(h0 - 3, h0 + 35)):
        if 0 <= g < H:
            xw[:, i] = x[:, g]
    xwin = np.ascontiguousarray(xw.reshape(2, 128, 38 * 64)).astype(bf)
    xT2 = np.zeros((128, 20, C), np.float32)
    for jh in range(2):
        base = h0 + 16 * jh - 2
        for i in range(20):
            g = base + i
            if 0 <= g < H:
                xT2[64 * jh:64 * jh + 64, i] = x[:, g].T
    xT2 = np.ascontiguousarray(xT2.reshape(128, 20 * 256)).astype(bf)
    w_comp = np.asarray(inputs['w_comp'], np.float32)[:, :, 0, 0]
    wc = np.zeros((2, 128, 64), np.float32)
    for cg in range(2):
        wc[cg] = w_comp[:, cg * 128:(cg + 1) * 128].T
    wc = np.ascontiguousarray(wc.transpose(1, 0, 2).reshape(128, 2 * 64)).astype(bf)
    w_ker = np.asarray(inputs['w_ker'], np.float32)
    w_off = np.asarray(inputs['w_off'], np.float32)
    # mask channels permuted to k = kx*5 + ky so the collate DMA reads a
    # contiguous 20-elem (ki, u) slice per horizontal tap kx
    kperm = np.array([ky * 5 + kx for kx in range(5) for ky in range(5)])
    wk = np.zeros((9, 64, 57), np.float32)
    for t in range(9):
        wk[t, :, 0:8] = w_off[:, :, t // 3, t % 3].T
        wk[t, :, 32:57] = w_ker[kperm, :, t // 3, t % 3].T
    wk = np.ascontiguousarray(wk.transpose(1, 0, 2).reshape(64, 9 * 57)).astype(bf)
    bcov = np.zeros((57, 1), np.float32)
    bcov[0:8, 0] = np.asarray(inputs['b_off'], np.float32)
    bcov[32:57, 0] = np.asarray(inputs['b_ker'], np.float32)[kperm]
    idx1, idx2 = _SCAT
    wv = np.tile(np.arange(64, dtype=np.float32), 2).reshape(128, 1)
    hr = (h0 + 16 * (np.arange(128)[:, None] // 64)
          + np.arange(16, dtype=np.float32)[None, :]).astype(np.float32)
    # shift matrices: shmat[pc, b*128+po] = 1 iff pc == po + (b-2), same 64-block
    sh = np.zeros((128, 5, 128), np.float16)
    for b in range(5):
        d = b - 2
        for po in range(128):
            pc = po + d
            if 0 <= pc < 128 and pc // 64 == po // 64:
                sh[pc, b, po] = 1.0
    return {
        'xwin': xwin, 'xT2': xT2, 'wc': wc, 'wk': wk, 'bco': bcov,
        'bcomp': np.asarray(inputs['b_comp'], np.float32).reshape(64, 1),
        'wvec': wv,
        'w63': (63.0 - wv).astype(np.float32),
        'hrow': np.ascontiguousarray(hr),
        'y63': np.ascontiguousarray(63.0 - hr),
        'ident': np.eye(128, dtype=np.float32),
        'shmat': np.ascontiguousarray(sh.reshape(128, 5 * 128)),
        'idx1': idx1, 'idx2': idx2,
    }


def kernel(**inputs):
    nc = _get_program()
    core_ids = list(range(8))
    in_maps = [_prep_core_inputs(inputs, cid // 2, cid % 2) for cid in core_ids]
    res = run_bass_kernel_spmd(nc, in_maps, core_ids)
    out = np.zeros((N, C, 128, 128), np.float32)
    for cid in core_ids:
        n, s = cid // 2, cid % 2
        op = np.asarray(res.results[cid]['outp']).reshape(256, 64, 128)
        out[n, :, s::2] = op
    return out


if __name__ == '__main__':
    d = np.load('/root/problem/ref_io.npz')
    inp = {k: d[k] for k in ('x', 'w_comp', 'b_comp', 'w_ker', 'b_ker', 'w_off', 'b_off')}
    out = kernel(**inp)
    ref = d['out']
    err = np.abs(out - ref).max()
    print('max abs err:', err, 'rel:', err / np.abs(ref).max())


# revision 20
# speedup vs baseline: 1.6134x; 1.1314x over previous
"""Trainium2 Bass kernel for nn_DLUPack (CARAFE-style dynamic upsampling).

Sharding: 8 cores = (batch n in [0,4)) x (output-row-parity s in {0,1});
core (n, s) computes low-res rows hh in [32s, 32s+32) -> all parity-s output rows.

Reference output mapping (its reshape scrambles positions):
  ref[n, c, 2y+i, 2x+j] = sum_k patches[c, hh, ww, k] * kern[hh, ww, k, u]
  with hh = 32s + 16jh + m:  row r = 8m + 2(ww//16) + s, col = 8*(ww%16) + 2u + jh.

Device pipeline per core (all post-conv tensors packed [128 = (jh, w)], FD halved
vs the [64, 2*FD] layout so DVE ops run in half the cycles):
  1. compressor 1x1 conv (PE, fp16) -> cx [64, 38, 66]
  2. offset+mask 3x3 convs (9 accumulated MMs) -> psum [57, .]: off ch 0-7, mask 32-56
     (mask channels host-permuted to k = kx*5 + ky order)
  3. exp in ACT evac; 20 PE-transposes of row-pairs (r, r+16) -> expT [128,20,25] f32
     softmax denom via free-dim reduce; expT4 = expT*recT bcast-u -> fp16 [128,20,100]
     partition-shifted copies expT4_{p1,m1} via SBUF-SBUF DMA (edges zeroed from zed)
  4. offset PE-transpose (row-pairs) -> deltT [128,16,8]; indicator bilinear W9 (DVE)
  5. kernc assembly [128, 16m, 25k, 4u]: 9 bcast-mult (1x) + 8 dense adds (2x)
  6. collate: 5 partition-shifted strided DMAs kernc -> data_all [128, 16, 100]
     (contaminated edge slots are skipped by the scatter idx tables = -1)
  7. per pair m: 2 local_scatter (GPSIMD) -> banded [128, 3x512 | 2x512]
  8. carafe: 5 accumulated MMs [128,128]x[128,512] per (pair, c-half) -> psum [128,512]
  9. ACT evac -> DMA out 4 contiguous output rows
"""
import sys
import numpy as np

sys.path.insert(0, '/opt/trn_rl_repo')

import ml_dtypes  # noqa: E402
from contextlib import ExitStack  # noqa: E402

import concourse.bass as bass  # noqa: E402
import concourse.tile as tile  # noqa: E402
from concourse import mybir, bacc  # noqa: E402
from concourse.bass_utils import run_bass_kernel_spmd  # noqa: E402

F32 = mybir.dt.float32
BF16 = mybir.dt.float16  # NOTE: fp16 (better mantissa), name kept for brevity
I16 = mybir.dt.int16
AF = mybir.ActivationFunctionType
OP = mybir.AluOpType

N, C, H, W = 4, 256, 64, 64


def _ap(base, off_elems, dims):
    return bass.AP(tensor=base.tensor, offset=base.offset + off_elems, ap=[list(d) for d in dims])


def build_scatter_tables():
    idx1 = -np.ones((128, 100), np.int16)
    idx2 = -np.ones((128, 100), np.int16)
    for p in range(128):
        jh, wpp = p // 64, p % 64
        for b in range(5):
            w = wpp + b - 2
            if not (0 <= w < 64):
                continue
            q, wl = w // 16, w % 16
            for ki in range(5):
                for u in range(4):
                    col = q * 128 + 8 * wl + 2 * u + jh
                    qidx = (b * 5 + ki) * 4 + u
                    if ki < 3:
                        idx1[p, qidx] = ki * 512 + col
                    else:
                        idx2[p, qidx] = (ki - 3) * 512 + col
    return idx1, idx2


def build_program():
    nc = bacc.Bacc(None, target_bir_lowering=False, debug=True)

    xwin = nc.declare_dram_parameter('xwin', [2, 128, 38 * 64], BF16, isOutput=False)
    xT2 = nc.declare_dram_parameter('xT2', [128, 20 * 256], BF16, isOutput=False)
    wc = nc.declare_dram_parameter('wc', [128, 2 * 64], BF16, isOutput=False)
    wk = nc.declare_dram_parameter('wk', [64, 9 * 57], BF16, isOutput=False)
    bco = nc.declare_dram_parameter('bco', [57, 1], F32, isOutput=False)
    bcomp = nc.declare_dram_parameter('bcomp', [64, 1], F32, isOutput=False)
    wvec = nc.declare_dram_parameter('wvec', [128, 1], F32, isOutput=False)
    w63 = nc.declare_dram_parameter('w63', [128, 1], F32, isOutput=False)
    hrow = nc.declare_dram_parameter('hrow', [128, 16], F32, isOutput=False)
    y63 = nc.declare_dram_parameter('y63', [128, 16], F32, isOutput=False)
    ident = nc.declare_dram_parameter('ident', [128, 128], F32, isOutput=False)
    shmat = nc.declare_dram_parameter('shmat', [128, 5 * 128], BF16, isOutput=False)
    idx1 = nc.declare_dram_parameter('idx1', [128, 100], I16, isOutput=False)
    idx2 = nc.declare_dram_parameter('idx2', [128, 100], I16, isOutput=False)
    outp = nc.declare_dram_parameter('outp', [256, 64 * 128], F32, isOutput=True)

    with tile.TileContext(nc) as tc, ExitStack() as ctx:
        sing = ctx.enter_context(tc.tile_pool(name='sing', bufs=1))
        work = ctx.enter_context(tc.tile_pool(name='work', bufs=1))
        band = ctx.enter_context(tc.tile_pool(name='band', bufs=4))
        rowp = ctx.enter_context(tc.tile_pool(name='rowp', bufs=4))
        psum = ctx.enter_context(tc.psum_pool(name='ps', bufs=3))
        psc = ctx.enter_context(tc.psum_pool(name='psc', bufs=3))
        ptr = ctx.enter_context(tc.psum_pool(name='ptr', bufs=1))

        def load(shape, dtype, src, eng=None):
            t = sing.tile(shape, dtype, name=f'ld_{src.tensor.name if hasattr(src, "tensor") else id(src)}')
            (eng or nc.sync).dma_start(out=t[:], in_=src[:])
            return t

        # small params first on sync queue so compute can start ASAP;
        # ident early (warm-up matmuls need it)
        wc_sb = load([128, 2, 64], BF16, wc)
        bcomp_sb = load([64, 1], F32, bcomp)
        id_sb = load([128, 128], F32, ident)
        # xwin loaded in row chunks alternating queues so compressor group g
        # starts after ~1/5 of the load
        xwin_sb = sing.tile([128, 2, 38 * 64], BF16)
        for ck in range(5):
            r0 = ck * 8
            rows = min(8, 38 - r0)
            for cg in range(2):
                eng = nc.sync if cg == 0 else nc.scalar
                eng.dma_start(
                    out=xwin_sb[:, cg, r0 * 64:(r0 + rows) * 64],
                    in_=_ap(xwin[:], cg * 128 * 2432 + r0 * 64,
                            [[2432, 128], [1, rows * 64]]))
        wk_sb = load([64, 9, 57], BF16, wk, eng=nc.gpsimd)
        bco_sb = load([57, 1], F32, bco, eng=nc.gpsimd)
        wvec_sb = load([128, 1], F32, wvec, eng=nc.gpsimd)
        w63_sb = load([128, 1], F32, w63, eng=nc.gpsimd)
        hrow_sb = load([128, 16], F32, hrow, eng=nc.gpsimd)
        y63_sb = load([128, 16], F32, y63, eng=nc.gpsimd)
        sh_sb = load([128, 5, 128], BF16, shmat, eng=nc.gpsimd)
        xT2_sb = load([128, 20 * 256], BF16, xT2, eng=nc.scalar)
        idx1_sb = load([128, 100], I16, idx1, eng=nc.scalar)
        idx2_sb = load([128, 100], I16, idx2, eng=nc.scalar)

        # PE warm-up: keep TensorE busy during input-DMA wait so HAM reaches 8/8
        pw = psc.tile([128, 512], F32, name='pcs_warm', tag='pcs')
        for _ in range(48):
            nc.tensor.matmul(pw[0:64, 0:64], id_sb[:, 0:64], id_sb[:, 0:64], start=True, stop=True)

        # ---- 1. compressor ----
        cx_sb = work.tile([64, 38, 66], BF16)
        nc.vector.memset(_ap(cx_sb[:], 0, [[38 * 66, 64], [66, 38], [1, 1]]), 0.0)
        nc.vector.memset(_ap(cx_sb[:], 65, [[38 * 66, 64], [66, 38], [1, 1]]), 0.0)
        for grp in range(5):
            g0 = grp * 8
            rows = min(8, 38 - g0)
            nn = rows * 64
            pcs = psum.tile([64, 512], F32)
            for cg in range(2):
                nc.tensor.matmul(pcs[:, :nn], wc_sb[:, cg, :],
                                 xwin_sb[:, cg, g0 * 64:g0 * 64 + nn],
                                 start=(cg == 0), stop=(cg == 1))
            nc.scalar.activation(
                out=_ap(cx_sb[:], g0 * 66 + 1, [[38 * 66, 64], [66, rows], [1, 64]]),
                in_=_ap(pcs[:], 0, [[512, 64], [64, rows], [1, 64]]),
                func=AF.Identity, bias=bcomp_sb[:], scale=1.0)

        # ---- 2-4. offset+mask convs, transposes, softmax, W9 -- emitted as an
        # interleaved pipeline so asm(G0)/scatters start right after conv grp1.
        # expS pair-interleaved (i, jh, w); conv row r = 16jh + i covers h = r-2
        expS = work.tile([25, 20, 2, 64], F32)
        offS = work.tile([8, 16, 2, 64], F32)
        expT = work.tile([128, 20, 25], F32)
        sumT = work.tile([128, 20], F32)
        recT = work.tile([128, 20], F32)
        expT4 = work.tile([128, 20, 25, 4], BF16)
        expT4_p1 = work.tile([128, 20, 25, 4], BF16)   # [p] = expT4[p+1]
        expT4_m1 = work.tile([128, 20, 25, 4], BF16)   # [p] = expT4[p-1]
        deltT = work.tile([128, 16, 8], BF16)
        pt = ptr.tile([128, 512], F32, name='pt_exp')
        po_t = ptr.tile([128, 512], F32, name='po_w')

        def emit_conv(grp, jh):
            i0 = grp * 5
            nn = 5 * 64
            pcs = psum.tile([57, 320], F32)
            for t in range(9):
                dy, dx = t // 3, t % 3
                rhs = _ap(cx_sb[:], (16 * jh + i0 + dy) * 66 + dx,
                          [[38 * 66, 64], [66, 5], [1, 64]])
                nc.tensor.matmul(pcs[:, :nn], wk_sb[:, t, :], rhs,
                                 start=(t == 0), stop=(t == 8))
            nc.scalar.activation(
                out=_ap(expS[:], (i0 * 2 + jh) * 64, [[20 * 128, 25], [128, 5], [1, 64]]),
                in_=_ap(pcs[:], 32 * 320, [[320, 25], [64, 5], [1, 64]]),
                func=AF.Exp, bias=bco_sb[32:57], scale=1.0)
            lo, hi = max(i0, 2), min(i0 + 5, 18)
            if lo < hi:
                nc.vector.tensor_scalar(
                    out=_ap(offS[:], ((lo - 2) * 2 + jh) * 64,
                            [[16 * 128, 8], [128, hi - lo], [1, 64]]),
                    in0=_ap(pcs[:], (lo - i0) * 64, [[320, 8], [64, hi - lo], [1, 64]]),
                    scalar1=bco_sb[0:8], scalar2=None, op0=OP.add)

        def emit_exp_tr(grp):
            for i in range(grp * 5, grp * 5 + 5):
                nc.tensor.transpose(pt[:, i * 25:i * 25 + 25],
                                    _ap(expS[:], i * 128, [[20 * 128, 25], [1, 128]]),
                                    id_sb[0:25, 0:25])
            nc.scalar.activation(
                out=_ap(expT[:], grp * 125, [[500, 128], [1, 125]]),
                in_=_ap(pt[:], grp * 125, [[512, 128], [1, 125]]),
                func=AF.Copy, scale=1.0)

        def emit_softmax(grp):
            g5 = grp * 5
            nc.vector.tensor_reduce(out=sumT[:, g5:g5 + 5], in_=expT[:, g5:g5 + 5, :],
                                    axis=mybir.AxisListType.X, op=OP.add)
            nc.vector.reciprocal(out=recT[:, g5:g5 + 5], in_=sumT[:, g5:g5 + 5])
            nc.vector.tensor_tensor(
                out=_ap(expT4[:], g5 * 100, [[2000, 128], [100, 5], [4, 25], [1, 4]]),
                in0=_ap(expT[:], g5 * 25, [[500, 128], [25, 5], [1, 25], [0, 4]]),
                in1=_ap(recT[:], g5, [[20, 128], [1, 5], [0, 25], [0, 4]]), op=OP.mult)

        def emit_shift(grp):
            # partition-shifted variants (w +- 1) via PE shift matmuls; shift
            # matrices zero block edges (SBUF-SBUF DMA shifts are ~13GB/s, avoid)
            for dst, sidx in ((expT4_p1, 3), (expT4_m1, 1)):
                psh = psc.tile([128, 512], F32, name=f'psh_{sidx}_{grp}', tag='pcs')
                nc.tensor.matmul(psh[:, 0:500], sh_sb[:, sidx, :],
                                 _ap(expT4[:], grp * 500, [[2000, 128], [1, 500]]),
                                 start=True, stop=True)
                nc.scalar.activation(
                    out=_ap(dst[:], grp * 500, [[2000, 128], [1, 500]]),
                    in_=_ap(psh[:], 0, [[512, 128], [1, 500]]),
                    func=AF.Copy, scale=1.0)

        def emit_off_tr(half):
            for m in range(half * 8, half * 8 + 8):
                nc.tensor.transpose(po_t[:, m * 8:m * 8 + 8],
                                    _ap(offS[:], m * 128, [[16 * 128, 8], [1, 128]]),
                                    id_sb[0:8, 0:8])
            nc.scalar.activation(
                out=_ap(deltT[:], half * 64, [[128, 128], [1, 64]]),
                in_=_ap(po_t[:], half * 64, [[512, 128], [1, 64]]),
                func=AF.Copy, scale=1.0)

        def wt(nm):
            return work.tile([128, 64], BF16, name=nm)

        t1, t2 = wt('t1'), wt('t2')
        gxc, x0r, wxt, omwx, x1r = wt('gxc'), wt('x0r'), wt('wxt'), wt('omwx'), wt('x1r')
        gyc, y0r, wyt, omwy, y1r = wt('gyc'), wt('y0r'), wt('wyt'), wt('omwy'), wt('y1r')
        ia, ib = wt('ia'), wt('ib')
        cwx = work.tile([128, 3, 64], BF16)
        rwy = work.tile([128, 3, 64], BF16)
        # W9 broadcast over ki: [t, m(16), ki(5), u(4)] so asm APs stay 3-free-dim
        W9b = work.tile([128, 9 * 320], BF16)

        def emit_w9(half):
            # whole indicator/bilinear chain on the m-slice [8h..8h+8)
            s = half * 32

            def sl(t):
                return _ap(t[:], s, [[64, 128], [1, 32]])

            def dview(chbase):
                return _ap(deltT[:], chbase + half * 64, [[128, 128], [8, 8], [1, 4]])

            def r4(t):
                return _ap(t[:], s, [[64, 128], [4, 8], [1, 4]])

            hrow_bc = _ap(hrow_sb[:], half * 8, [[16, 128], [1, 8], [0, 4]])
            y63_bc = _ap(y63_sb[:], half * 8, [[16, 128], [1, 8], [0, 4]])
            ts, tt = nc.vector.tensor_scalar, nc.vector.tensor_tensor
            ts(out=sl(t1), in0=dview(0), scalar1=wvec_sb[:], scalar2=None, op0=OP.add)
            ts(out=sl(t2), in0=sl(t1), scalar1=0.0, scalar2=63.0, op0=OP.max, op1=OP.min)
            ts(out=sl(gxc), in0=sl(t2), scalar1=wvec_sb[:], scalar2=None, op0=OP.subtract)
            ts(out=sl(x0r), in0=sl(gxc), scalar1=0.0, scalar2=-1.0, op0=OP.is_lt, op1=OP.mult)
            tt(out=sl(wxt), in0=sl(gxc), in1=sl(x0r), op=OP.subtract)
            ts(out=sl(omwx), in0=sl(wxt), scalar1=-1.0, scalar2=1.0, op0=OP.mult, op1=OP.add)
            ts(out=sl(x1r), in0=sl(x0r), scalar1=1.0, scalar2=w63_sb[:], op0=OP.add, op1=OP.min)

            tt(out=r4(t1), in0=dview(4), in1=hrow_bc, op=OP.add)
            ts(out=sl(t2), in0=sl(t1), scalar1=0.0, scalar2=63.0, op0=OP.max, op1=OP.min)
            tt(out=r4(gyc), in0=r4(t2), in1=hrow_bc, op=OP.subtract)
            ts(out=sl(y0r), in0=sl(gyc), scalar1=0.0, scalar2=-1.0, op0=OP.is_lt, op1=OP.mult)
            tt(out=sl(wyt), in0=sl(gyc), in1=sl(y0r), op=OP.subtract)
            ts(out=sl(omwy), in0=sl(wyt), scalar1=-1.0, scalar2=1.0, op0=OP.mult, op1=OP.add)
            ts(out=sl(t1), in0=sl(y0r), scalar1=1.0, scalar2=None, op0=OP.add)
            tt(out=r4(y1r), in0=r4(t1), in1=y63_bc, op=OP.min)

            for i, e in enumerate((-1.0, 0.0, 1.0)):
                ts(out=sl(ia), in0=sl(x0r), scalar1=e, scalar2=None, op0=OP.is_equal)
                ts(out=sl(ib), in0=sl(x1r), scalar1=e, scalar2=None, op0=OP.is_equal)
                tt(out=sl(ia), in0=sl(ia), in1=sl(omwx), op=OP.mult)
                tt(out=sl(ib), in0=sl(ib), in1=sl(wxt), op=OP.mult)
                tt(out=_ap(cwx[:], i * 64 + s, [[192, 128], [1, 32]]),
                   in0=sl(ia), in1=sl(ib), op=OP.add)
                ts(out=sl(ia), in0=sl(y0r), scalar1=e, scalar2=None, op0=OP.is_equal)
                ts(out=sl(ib), in0=sl(y1r), scalar1=e, scalar2=None, op0=OP.is_equal)
                tt(out=sl(ia), in0=sl(ia), in1=sl(omwy), op=OP.mult)
                tt(out=sl(ib), in0=sl(ib), in1=sl(wyt), op=OP.mult)
                tt(out=_ap(rwy[:], i * 64 + s, [[192, 128], [1, 32]]),
                   in0=sl(ia), in1=sl(ib), op=OP.add)
            for iy in range(3):
                for ix in range(3):
                    tt(out=_ap(W9b[:], (iy * 3 + ix) * 320 + half * 160,
                               [[9 * 320, 128], [20, 8], [4, 5], [1, 4]]),
                       in0=_ap(rwy[:], iy * 64 + s, [[192, 128], [4, 8], [0, 5], [1, 4]]),
                       in1=_ap(cwx[:], ix * 64 + s, [[192, 128], [4, 8], [0, 5], [1, 4]]),
                       op=OP.mult)

        # ---- 5-9 software-pipelined by m-groups ----
        # kernc layout [p, kx(5), m(16), ki(5), u(4)] so per-(kx, m-group) slices
        # are contiguous for the collate shift-matmul rhs
        kernc = work.tile([128, 5, 16, 5, 4], BF16)
        tmp = work.tile([128, 400], BF16)
        data_all = work.tile([128, 16, 100], BF16)
        exp_by_ex = {-1: expT4_m1, 0: expT4, 1: expT4_p1}
        NG = 4
        GM = 16 // NG

        def emit_asm(G):
            kv = _ap(kernc[:], GM * G * 20, [[1600, 128], [320, 5], [20, GM], [1, 20]])
            tv = _ap(tmp[:], 0, [[400, 128], [80, 5], [20, GM], [1, 20]])
            first = True
            for iy, ey in enumerate((-1, 0, 1)):
                for ix, ex in enumerate((-1, 0, 1)):
                    # expT4 [row(20), kx(5), ki(5), u(4)] -> read as [kx, m, kiu]
                    mv = _ap(exp_by_ex[ex][:], (2 + ey + GM * G) * 100,
                             [[2000, 128], [20, 5], [100, GM], [1, 20]])
                    wv = _ap(W9b[:], (iy * 3 + ix) * 320 + GM * G * 20,
                             [[9 * 320, 128], [0, 5], [20, GM], [1, 20]])
                    if first:
                        nc.vector.tensor_tensor(out=kv, in0=wv, in1=mv, op=OP.mult)
                        first = False
                    else:
                        nc.vector.tensor_tensor(out=tv, in0=wv, in1=mv, op=OP.mult)
                        nc.vector.tensor_tensor(out=kv, in0=kv, in1=tv, op=OP.add)

        def emit_collate(G):
            # data_all[po, m, b*20+(ki,u)] = kernc[po+(b-2), kx=4-b, m, ki, u]
            # via PE shift matmuls (zero-padded at block edges by the matrices)
            pda = psc.tile([128, 512], F32, name=f'pda_{G}', tag='pcs')
            for b in range(5):
                rhs = _ap(kernc[:], (4 - b) * 320 + GM * G * 20, [[1600, 128], [1, 80]])
                nc.tensor.matmul(pda[:, b * 80:b * 80 + 80], sh_sb[:, b, :], rhs,
                                 start=True, stop=True)
            nc.scalar.activation(
                out=_ap(data_all[:], GM * G * 100, [[1600, 128], [100, GM], [20, 5], [1, 20]]),
                in_=_ap(pda[:], 0, [[512, 128], [20, GM], [80, 5], [1, 20]]),
                func=AF.Copy, scale=1.0)

        def emit_pairs(G):
            for m in range(GM * G, GM * G + GM):
                banded1 = band.tile([128, 1536], BF16, name=f'band1_{m}', tag='band1')
                banded2 = band.tile([128, 1024], BF16, name=f'band2_{m}', tag='band2')
                nc.gpsimd.local_scatter(out_ap=banded1[:], data_ap=data_all[:, m, :],
                                        idxs_ap=idx1_sb[:], channels=128, num_elems=1536, num_idxs=100)
                nc.gpsimd.local_scatter(out_ap=banded2[:], data_ap=data_all[:, m, :],
                                        idxs_ap=idx2_sb[:], channels=128, num_elems=1024, num_idxs=100)
                for ch in range(2):
                    pcs = psc.tile([128, 512], F32, name=f'pcs_{m}_{ch}', tag='pcs')
                    for ki in range(5):
                        lhsT = _ap(xT2_sb[:], (m + ki) * 256 + ch * 128, [[20 * 256, 128], [1, 128]])
                        rhs = banded1[:, ki * 512:ki * 512 + 512] if ki < 3 \
                            else banded2[:, (ki - 3) * 512:(ki - 3) * 512 + 512]
                        nc.tensor.matmul(pcs[:], lhsT, rhs, start=(ki == 0), stop=(ki == 4))
                    rb = rowp.tile([128, 512], F32, name=f'rb_{m}_{ch}', tag='rb')
                    nc.scalar.activation(out=rb[:], in_=pcs[:], func=AF.Copy, scale=1.0)
                    nc.sync.dma_start(
                        out=_ap(outp[:], ch * 128 * 8192 + 4 * m * 128,
                                [[8192, 128], [128, 4], [1, 128]]),
                        in_=rb[:])

        # interleaved emission: per-engine queues are in-order, so collates
        # (PE, dependent on DVE asm) are placed after the conv groups they
        # must not delay; scatters start right after asm(G0)+collate(G0)
        emit_conv(0, 0); emit_conv(0, 1); emit_exp_tr(0); emit_softmax(0)
        emit_conv(1, 0); emit_conv(1, 1); emit_exp_tr(1); emit_softmax(1)
        emit_off_tr(0); emit_shift(0); emit_shift(1); emit_w9(0)
        emit_conv(2, 0); emit_conv(2, 1); emit_exp_tr(2); emit_softmax(2)
        emit_shift(2); emit_asm(0); emit_collate(0)
        emit_conv(3, 0); emit_conv(3, 1); emit_exp_tr(3); emit_off_tr(1)
        emit_softmax(3); emit_shift(3); emit_w9(1); emit_asm(1); emit_collate(1)
        emit_pairs(0); emit_asm(2); emit_collate(2)
        emit_pairs(1); emit_asm(3); emit_collate(3)
        emit_pairs(2)
        emit_pairs(3)
    nc.finalize()
    return nc


_PROGRAM = None
_SCAT = build_scatter_tables()


def _get_program():
    global _PROGRAM
    if _PROGRAM is None:
        _PROGRAM = build_program()
    return _PROGRAM


def _prep_core_inputs(inputs, n, s):
    bf = np.float16
    x = np.asarray(inputs['x'][n], np.float32)
    h0 = 32 * s
    xw = np.zeros((C, 38, W), np.float32)
    for i, g in enumerate(range(h0 - 3, h0 + 35)):
        if 0 <= g < H:
            xw[:, i] = x[:, g]
    xwin = np.ascontiguousarray(xw.reshape(2, 128, 38 * 64)).astype(bf)
    xT2 = np.zeros((128, 20, C), np.float32)
    for jh in range(2):
        base = h0 + 16 * jh - 2
        for i in range(20):
            g = base + i
            if 0 <= g < H:
                xT2[64 * jh:64 * jh + 64, i] = x[:, g].T
    xT2 = np.ascontiguousarray(xT2.reshape(128, 20 * 256)).astype(bf)
    w_comp = np.asarray(inputs['w_comp'], np.float32)[:, :, 0, 0]
    wc = np.zeros((2, 128, 64), np.float32)
    for cg in range(2):
        wc[cg] = w_comp[:, cg * 128:(cg + 1) * 128].T
    wc = np.ascontiguousarray(wc.transpose(1, 0, 2).reshape(128, 2 * 64)).astype(bf)
    w_ker = np.asarray(inputs['w_ker'], np.float32)
    w_off = np.asarray(inputs['w_off'], np.float32)
    # mask channels permuted to k = kx*5 + ky so the collate DMA reads a
    # contiguous 20-elem (ki, u) slice per horizontal tap kx
    kperm = np.array([ky * 5 + kx for kx in range(5) for ky in range(5)])
    wk = np.zeros((9, 64, 57), np.float32)
    for t in range(9):
        wk[t, :, 0:8] = w_off[:, :, t // 3, t % 3].T
        wk[t, :, 32:57] = w_ker[kperm, :, t // 3, t % 3].T
    wk = np.ascontiguousarray(wk.transpose(1, 0, 2).reshape(64, 9 * 57)).astype(bf)
    bcov = np.zeros((57, 1), np.float32)
    bcov[0:8, 0] = np.asarray(inputs['b_off'], np.float32)
    bcov[32:57, 0] = np.asarray(inputs['b_ker'], np.float32)[kperm]
    idx1, idx2 = _SCAT
    wv = np.tile(np.arange(64, dtype=np.float32), 2).reshape(128, 1)
    hr = (h0 + 16 * (np.arange(128)[:, None] // 64)
          + np.arange(16, dtype=np.float32)[None, :]).astype(np.float32)
    # shift matrices: shmat[pc, b*128+po] = 1 iff pc == po + (b-2), same 64-block
    sh = np.zeros((128, 5, 128), np.float16)
    for b in range(5):
        d = b - 2
        for po in range(128):
            pc = po + d
            if 0 <= pc < 128 and pc // 64 == po // 64:
                sh[pc, b, po] = 1.0
    return {
        'xwin': xwin, 'xT2': xT2, 'wc': wc, 'wk': wk, 'bco': bcov,
        'bcomp': np.asarray(inputs['b_comp'], np.float32).reshape(64, 1),
        'wvec': wv,
        'w63': (63.0 - wv).astype(np.float32),
        'hrow': np.ascontiguousarray(hr),
        'y63': np.ascontiguousarray(63.0 - hr),
        'ident': np.eye(128, dtype=np.float32),
        'shmat': np.ascontiguousarray(sh.reshape(128, 5 * 128)),
        'idx1': idx1, 'idx2': idx2,
    }


def kernel(**inputs):
    nc = _get_program()
    core_ids = list(range(8))
    in_maps = [_prep_core_inputs(inputs, cid // 2, cid % 2) for cid in core_ids]
    res = run_bass_kernel_spmd(nc, in_maps, core_ids)
    out = np.zeros((N, C, 128, 128), np.float32)
    for cid in core_ids:
        n, s = cid // 2, cid % 2
        op = np.asarray(res.results[cid]['outp']).reshape(256, 64, 128)
        out[n, :, s::2] = op
    return out


if __name__ == '__main__':
    d = np.load('/root/problem/ref_io.npz')
    inp = {k: d[k] for k in ('x', 'w_comp', 'b_comp', 'w_ker', 'b_ker', 'w_off', 'b_off')}
    out = kernel(**inp)
    ref = d['out']
    err = np.abs(out - ref).max()
    print('max abs err:', err, 'rel:', err / np.abs(ref).max())


# revision 23
# speedup vs baseline: 1.6753x; 1.0384x over previous
"""Trainium2 Bass kernel for nn_DLUPack (CARAFE-style dynamic upsampling).

Sharding: 8 cores = (batch n in [0,4)) x (output-row-parity s in {0,1});
core (n, s) computes low-res rows hh in [32s, 32s+32) -> all parity-s output rows.

Reference output mapping (its reshape scrambles positions):
  ref[n, c, 2y+i, 2x+j] = sum_k patches[c, hh, ww, k] * kern[hh, ww, k, u]
  with hh = 32s + 16jh + m:  row r = 8m + 2(ww//16) + s, col = 8*(ww%16) + 2u + jh.

Device pipeline per core (all post-conv tensors packed [128 = (jh, w)], FD halved
vs the [64, 2*FD] layout so DVE ops run in half the cycles):
  1. compressor 1x1 conv (PE, fp16) -> cx [64, 38, 66]
  2. offset+mask 3x3 convs (9 accumulated MMs) -> psum [57, .]: off ch 0-7, mask 32-56
     (mask channels host-permuted to k = kx*5 + ky order)
  3. exp in ACT evac; 20 PE-transposes of row-pairs (r, r+16) -> expT [128,20,25] f32
     softmax denom via free-dim reduce; expT4 = expT*recT bcast-u -> fp16 [128,20,100]
     partition-shifted copies expT4_{p1,m1} via SBUF-SBUF DMA (edges zeroed from zed)
  4. offset PE-transpose (row-pairs) -> deltT [128,16,8]; indicator bilinear W9 (DVE)
  5. kernc assembly [128, 16m, 25k, 4u]: 9 bcast-mult (1x) + 8 dense adds (2x)
  6. collate: 5 partition-shifted strided DMAs kernc -> data_all [128, 16, 100]
     (contaminated edge slots are skipped by the scatter idx tables = -1)
  7. per pair m: 2 local_scatter (GPSIMD) -> banded [128, 3x512 | 2x512]
  8. carafe: 5 accumulated MMs [128,128]x[128,512] per (pair, c-half) -> psum [128,512]
  9. ACT evac -> DMA out 4 contiguous output rows
"""
import sys
import numpy as np

sys.path.insert(0, '/opt/trn_rl_repo')

import ml_dtypes  # noqa: E402
from contextlib import ExitStack  # noqa: E402

import concourse.bass as bass  # noqa: E402
import concourse.tile as tile  # noqa: E402
from concourse import mybir, bacc  # noqa: E402
from concourse.bass_utils import run_bass_kernel_spmd  # noqa: E402

F32 = mybir.dt.float32
BF16 = mybir.dt.float16  # NOTE: fp16 (better mantissa), name kept for brevity
I16 = mybir.dt.int16
AF = mybir.ActivationFunctionType
OP = mybir.AluOpType

N, C, H, W = 4, 256, 64, 64


def _ap(base, off_elems, dims):
    return bass.AP(tensor=base.tensor, offset=base.offset + off_elems, ap=[list(d) for d in dims])


def build_scatter_tables():
    idx1 = -np.ones((128, 100), np.int16)
    idx2 = -np.ones((128, 100), np.int16)
    for p in range(128):
        jh, wpp = p // 64, p % 64
        for b in range(5):
            w = wpp + b - 2
            if not (0 <= w < 64):
                continue
            q, wl = w // 16, w % 16
            for ki in range(5):
                for u in range(4):
                    col = q * 128 + 8 * wl + 2 * u + jh
                    qidx = (b * 5 + ki) * 4 + u
                    if ki < 3:
                        idx1[p, qidx] = ki * 512 + col
                    else:
                        idx2[p, qidx] = (ki - 3) * 512 + col
    return idx1, idx2


def build_program():
    nc = bacc.Bacc(None, target_bir_lowering=False, debug=True)

    xwin = nc.declare_dram_parameter('xwin', [2, 128, 38 * 64], BF16, isOutput=False)
    xT2 = nc.declare_dram_parameter('xT2', [128, 20 * 256], BF16, isOutput=False)
    wc = nc.declare_dram_parameter('wc', [128, 2 * 64], BF16, isOutput=False)
    wk = nc.declare_dram_parameter('wk', [64, 9 * 57], BF16, isOutput=False)
    bco = nc.declare_dram_parameter('bco', [57, 1], F32, isOutput=False)
    bcomp = nc.declare_dram_parameter('bcomp', [64, 1], F32, isOutput=False)
    wvec = nc.declare_dram_parameter('wvec', [128, 1], F32, isOutput=False)
    w63 = nc.declare_dram_parameter('w63', [128, 1], F32, isOutput=False)
    hrow = nc.declare_dram_parameter('hrow', [128, 16], F32, isOutput=False)
    y63 = nc.declare_dram_parameter('y63', [128, 16], F32, isOutput=False)
    ident = nc.declare_dram_parameter('ident', [128, 128], F32, isOutput=False)
    shmat = nc.declare_dram_parameter('shmat', [128, 5 * 128], BF16, isOutput=False)
    idx1 = nc.declare_dram_parameter('idx1', [128, 100], I16, isOutput=False)
    idx2 = nc.declare_dram_parameter('idx2', [128, 100], I16, isOutput=False)
    outp = nc.declare_dram_parameter('outp', [256, 64 * 128], F32, isOutput=True)

    with tile.TileContext(nc) as tc, ExitStack() as ctx:
        sing = ctx.enter_context(tc.tile_pool(name='sing', bufs=1))
        work = ctx.enter_context(tc.tile_pool(name='work', bufs=1))
        band = ctx.enter_context(tc.tile_pool(name='band', bufs=6))
        rowp = ctx.enter_context(tc.tile_pool(name='rowp', bufs=4))
        psum = ctx.enter_context(tc.psum_pool(name='ps', bufs=3))
        psc = ctx.enter_context(tc.psum_pool(name='psc', bufs=3))
        ptr = ctx.enter_context(tc.psum_pool(name='ptr', bufs=1))

        def load(shape, dtype, src, eng=None):
            t = sing.tile(shape, dtype, name=f'ld_{src.tensor.name if hasattr(src, "tensor") else id(src)}')
            (eng or nc.sync).dma_start(out=t[:], in_=src[:])
            return t

        # small params first on sync queue so compute can start ASAP;
        # ident early (warm-up matmuls need it)
        wc_sb = load([128, 2, 64], BF16, wc)
        bcomp_sb = load([64, 1], F32, bcomp)
        id_sb = load([128, 128], F32, ident)
        # xwin loaded in row chunks alternating queues so compressor group g
        # starts after ~1/5 of the load
        xwin_sb = sing.tile([128, 2, 38 * 64], BF16)
        for ck in range(5):
            r0 = ck * 8
            rows = min(8, 38 - r0)
            for cg in range(2):
                eng = nc.sync if cg == 0 else nc.scalar
                eng.dma_start(
                    out=xwin_sb[:, cg, r0 * 64:(r0 + rows) * 64],
                    in_=_ap(xwin[:], cg * 128 * 2432 + r0 * 64,
                            [[2432, 128], [1, rows * 64]]))
        wk_sb = load([64, 9, 57], BF16, wk, eng=nc.gpsimd)
        bco_sb = load([57, 1], F32, bco, eng=nc.gpsimd)
        wvec_sb = load([128, 1], F32, wvec, eng=nc.gpsimd)
        w63_sb = load([128, 1], F32, w63, eng=nc.gpsimd)
        hrow_sb = load([128, 16], F32, hrow, eng=nc.gpsimd)
        y63_sb = load([128, 16], F32, y63, eng=nc.gpsimd)
        sh_sb = load([128, 5, 128], BF16, shmat, eng=nc.gpsimd)
        xT2_sb = load([128, 20 * 256], BF16, xT2, eng=nc.scalar)
        idx1_sb = load([128, 100], I16, idx1, eng=nc.scalar)
        idx2_sb = load([128, 100], I16, idx2, eng=nc.scalar)

        # PE warm-up: keep TensorE busy during input-DMA wait so HAM reaches 8/8
        pw = psc.tile([128, 512], F32, name='pcs_warm', tag='pcs')
        for _ in range(48):
            nc.tensor.matmul(pw[0:64, 0:64], id_sb[:, 0:64], id_sb[:, 0:64], start=True, stop=True)

        # ---- 1. compressor ----
        cx_sb = work.tile([64, 38, 66], BF16)
        nc.vector.memset(_ap(cx_sb[:], 0, [[38 * 66, 64], [66, 38], [1, 1]]), 0.0)
        nc.vector.memset(_ap(cx_sb[:], 65, [[38 * 66, 64], [66, 38], [1, 1]]), 0.0)
        for grp in range(5):
            g0 = grp * 8
            rows = min(8, 38 - g0)
            nn = rows * 64
            pcs = psum.tile([64, 512], F32)
            for cg in range(2):
                nc.tensor.matmul(pcs[:, :nn], wc_sb[:, cg, :],
                                 xwin_sb[:, cg, g0 * 64:g0 * 64 + nn],
                                 start=(cg == 0), stop=(cg == 1))
            nc.scalar.activation(
                out=_ap(cx_sb[:], g0 * 66 + 1, [[38 * 66, 64], [66, rows], [1, 64]]),
                in_=_ap(pcs[:], 0, [[512, 64], [64, rows], [1, 64]]),
                func=AF.Identity, bias=bcomp_sb[:], scale=1.0)

        # ---- 2-4. offset+mask convs, transposes, softmax, W9 -- emitted as an
        # interleaved pipeline so asm(G0)/scatters start right after conv grp1.
        # expS pair-interleaved (i, jh, w); conv row r = 16jh + i covers h = r-2
        expS = work.tile([25, 20, 2, 64], F32)
        offS = work.tile([8, 16, 2, 64], F32)
        expT = work.tile([128, 20, 25], F32)
        sumT = work.tile([128, 20], F32)
        recT = work.tile([128, 20], F32)
        expT4 = work.tile([128, 20, 25, 4], BF16)
        expT4_p1 = work.tile([128, 20, 25, 4], BF16)   # [p] = expT4[p+1]
        expT4_m1 = work.tile([128, 20, 25, 4], BF16)   # [p] = expT4[p-1]
        deltT = work.tile([128, 16, 8], BF16)
        pt = ptr.tile([128, 512], F32, name='pt_exp')
        po_t = ptr.tile([128, 512], F32, name='po_w')

        def emit_conv(grp, jh):
            i0 = grp * 5
            nn = 5 * 64
            pcs = psum.tile([57, 320], F32)
            for t in range(9):
                dy, dx = t // 3, t % 3
                rhs = _ap(cx_sb[:], (16 * jh + i0 + dy) * 66 + dx,
                          [[38 * 66, 64], [66, 5], [1, 64]])
                nc.tensor.matmul(pcs[:, :nn], wk_sb[:, t, :], rhs,
                                 start=(t == 0), stop=(t == 8))
            nc.scalar.activation(
                out=_ap(expS[:], (i0 * 2 + jh) * 64, [[20 * 128, 25], [128, 5], [1, 64]]),
                in_=_ap(pcs[:], 32 * 320, [[320, 25], [64, 5], [1, 64]]),
                func=AF.Exp, bias=bco_sb[32:57], scale=1.0)
            lo, hi = max(i0, 2), min(i0 + 5, 18)
            if lo < hi:
                nc.vector.tensor_scalar(
                    out=_ap(offS[:], ((lo - 2) * 2 + jh) * 64,
                            [[16 * 128, 8], [128, hi - lo], [1, 64]]),
                    in0=_ap(pcs[:], (lo - i0) * 64, [[320, 8], [64, hi - lo], [1, 64]]),
                    scalar1=bco_sb[0:8], scalar2=None, op0=OP.add)

        def emit_exp_tr(grp):
            for i in range(grp * 5, grp * 5 + 5):
                nc.tensor.transpose(pt[:, i * 25:i * 25 + 25],
                                    _ap(expS[:], i * 128, [[20 * 128, 25], [1, 128]]),
                                    id_sb[0:25, 0:25])
            nc.scalar.activation(
                out=_ap(expT[:], grp * 125, [[500, 128], [1, 125]]),
                in_=_ap(pt[:], grp * 125, [[512, 128], [1, 125]]),
                func=AF.Copy, scale=1.0)

        def emit_softmax(grp):
            g5 = grp * 5
            nc.vector.tensor_reduce(out=sumT[:, g5:g5 + 5], in_=expT[:, g5:g5 + 5, :],
                                    axis=mybir.AxisListType.X, op=OP.add)
            nc.vector.reciprocal(out=recT[:, g5:g5 + 5], in_=sumT[:, g5:g5 + 5])
            nc.vector.tensor_tensor(
                out=_ap(expT4[:], g5 * 100, [[2000, 128], [100, 5], [4, 25], [1, 4]]),
                in0=_ap(expT[:], g5 * 25, [[500, 128], [25, 5], [1, 25], [0, 4]]),
                in1=_ap(recT[:], g5, [[20, 128], [1, 5], [0, 25], [0, 4]]), op=OP.mult)

        def emit_shift(grp):
            # partition-shifted variants (w +- 1) via PE shift matmuls; shift
            # matrices zero block edges (SBUF-SBUF DMA shifts are ~13GB/s, avoid)
            for dst, sidx in ((expT4_p1, 3), (expT4_m1, 1)):
                psh = psc.tile([128, 512], F32, name=f'psh_{sidx}_{grp}', tag='pcs')
                nc.tensor.matmul(psh[:, 0:500], sh_sb[:, sidx, :],
                                 _ap(expT4[:], grp * 500, [[2000, 128], [1, 500]]),
                                 start=True, stop=True)
                nc.scalar.activation(
                    out=_ap(dst[:], grp * 500, [[2000, 128], [1, 500]]),
                    in_=_ap(psh[:], 0, [[512, 128], [1, 500]]),
                    func=AF.Copy, scale=1.0)

        def emit_off_tr(half):
            for m in range(half * 8, half * 8 + 8):
                nc.tensor.transpose(po_t[:, m * 8:m * 8 + 8],
                                    _ap(offS[:], m * 128, [[16 * 128, 8], [1, 128]]),
                                    id_sb[0:8, 0:8])
            nc.scalar.activation(
                out=_ap(deltT[:], half * 64, [[128, 128], [1, 64]]),
                in_=_ap(po_t[:], half * 64, [[512, 128], [1, 64]]),
                func=AF.Copy, scale=1.0)

        def wt(nm):
            return work.tile([128, 64], BF16, name=nm)

        t1, t2 = wt('t1'), wt('t2')
        gxc, x0r, wxt, omwx, x1r = wt('gxc'), wt('x0r'), wt('wxt'), wt('omwx'), wt('x1r')
        gyc, y0r, wyt, omwy, y1r = wt('gyc'), wt('y0r'), wt('wyt'), wt('omwy'), wt('y1r')
        ia, ib = wt('ia'), wt('ib')
        cwx = work.tile([128, 3, 64], BF16)
        rwy = work.tile([128, 3, 64], BF16)
        # W9 broadcast over ki: [t, m(16), ki(5), u(4)] so asm APs stay 3-free-dim
        W9b = work.tile([128, 9 * 320], BF16)

        def emit_w9(half):
            # whole indicator/bilinear chain on the m-slice [8h..8h+8)
            s = half * 32

            def sl(t):
                return _ap(t[:], s, [[64, 128], [1, 32]])

            def dview(chbase):
                return _ap(deltT[:], chbase + half * 64, [[128, 128], [8, 8], [1, 4]])

            def r4(t):
                return _ap(t[:], s, [[64, 128], [4, 8], [1, 4]])

            hrow_bc = _ap(hrow_sb[:], half * 8, [[16, 128], [1, 8], [0, 4]])
            y63_bc = _ap(y63_sb[:], half * 8, [[16, 128], [1, 8], [0, 4]])
            ts, tt = nc.vector.tensor_scalar, nc.vector.tensor_tensor
            ts(out=sl(t1), in0=dview(0), scalar1=wvec_sb[:], scalar2=None, op0=OP.add)
            ts(out=sl(t2), in0=sl(t1), scalar1=0.0, scalar2=63.0, op0=OP.max, op1=OP.min)
            ts(out=sl(gxc), in0=sl(t2), scalar1=wvec_sb[:], scalar2=None, op0=OP.subtract)
            ts(out=sl(x0r), in0=sl(gxc), scalar1=0.0, scalar2=-1.0, op0=OP.is_lt, op1=OP.mult)
            tt(out=sl(wxt), in0=sl(gxc), in1=sl(x0r), op=OP.subtract)
            ts(out=sl(omwx), in0=sl(wxt), scalar1=-1.0, scalar2=1.0, op0=OP.mult, op1=OP.add)
            ts(out=sl(x1r), in0=sl(x0r), scalar1=1.0, scalar2=w63_sb[:], op0=OP.add, op1=OP.min)

            tt(out=r4(t1), in0=dview(4), in1=hrow_bc, op=OP.add)
            ts(out=sl(t2), in0=sl(t1), scalar1=0.0, scalar2=63.0, op0=OP.max, op1=OP.min)
            tt(out=r4(gyc), in0=r4(t2), in1=hrow_bc, op=OP.subtract)
            ts(out=sl(y0r), in0=sl(gyc), scalar1=0.0, scalar2=-1.0, op0=OP.is_lt, op1=OP.mult)
            tt(out=sl(wyt), in0=sl(gyc), in1=sl(y0r), op=OP.subtract)
            ts(out=sl(omwy), in0=sl(wyt), scalar1=-1.0, scalar2=1.0, op0=OP.mult, op1=OP.add)
            ts(out=sl(t1), in0=sl(y0r), scalar1=1.0, scalar2=None, op0=OP.add)
            tt(out=r4(y1r), in0=r4(t1), in1=y63_bc, op=OP.min)

            for i, e in enumerate((-1.0, 0.0, 1.0)):
                ts(out=sl(ia), in0=sl(x0r), scalar1=e, scalar2=None, op0=OP.is_equal)
                ts(out=sl(ib), in0=sl(x1r), scalar1=e, scalar2=None, op0=OP.is_equal)
                tt(out=sl(ia), in0=sl(ia), in1=sl(omwx), op=OP.mult)
                tt(out=sl(ib), in0=sl(ib), in1=sl(wxt), op=OP.mult)
                tt(out=_ap(cwx[:], i * 64 + s, [[192, 128], [1, 32]]),
                   in0=sl(ia), in1=sl(ib), op=OP.add)
                ts(out=sl(ia), in0=sl(y0r), scalar1=e, scalar2=None, op0=OP.is_equal)
                ts(out=sl(ib), in0=sl(y1r), scalar1=e, scalar2=None, op0=OP.is_equal)
                tt(out=sl(ia), in0=sl(ia), in1=sl(omwy), op=OP.mult)
                tt(out=sl(ib), in0=sl(ib), in1=sl(wyt), op=OP.mult)
                tt(out=_ap(rwy[:], i * 64 + s, [[192, 128], [1, 32]]),
                   in0=sl(ia), in1=sl(ib), op=OP.add)
            for iy in range(3):
                for ix in range(3):
                    tt(out=_ap(W9b[:], (iy * 3 + ix) * 320 + half * 160,
                               [[9 * 320, 128], [20, 8], [4, 5], [1, 4]]),
                       in0=_ap(rwy[:], iy * 64 + s, [[192, 128], [4, 8], [0, 5], [1, 4]]),
                       in1=_ap(cwx[:], ix * 64 + s, [[192, 128], [4, 8], [0, 5], [1, 4]]),
                       op=OP.mult)

        # ---- 5-9 software-pipelined by m-groups ----
        # kernc layout [p, kx(5), m(16), ki(5), u(4)] so per-(kx, m-group) slices
        # are contiguous for the collate shift-matmul rhs
        kernc = work.tile([128, 5, 16, 5, 4], BF16)
        tmp = work.tile([128, 400], BF16)
        data_all = work.tile([128, 16, 100], BF16)
        exp_by_ex = {-1: expT4_m1, 0: expT4, 1: expT4_p1}
        NG = 4
        GM = 16 // NG

        def emit_asm(G):
            kv = _ap(kernc[:], GM * G * 20, [[1600, 128], [320, 5], [20, GM], [1, 20]])
            tv = _ap(tmp[:], 0, [[400, 128], [80, 5], [20, GM], [1, 20]])
            first = True
            for iy, ey in enumerate((-1, 0, 1)):
                for ix, ex in enumerate((-1, 0, 1)):
                    # expT4 [row(20), kx(5), ki(5), u(4)] -> read as [kx, m, kiu]
                    mv = _ap(exp_by_ex[ex][:], (2 + ey + GM * G) * 100,
                             [[2000, 128], [20, 5], [100, GM], [1, 20]])
                    wv = _ap(W9b[:], (iy * 3 + ix) * 320 + GM * G * 20,
                             [[9 * 320, 128], [0, 5], [20, GM], [1, 20]])
                    if first:
                        nc.vector.tensor_tensor(out=kv, in0=wv, in1=mv, op=OP.mult)
                        first = False
                    else:
                        nc.vector.tensor_tensor(out=tv, in0=wv, in1=mv, op=OP.mult)
                        nc.vector.tensor_tensor(out=kv, in0=kv, in1=tv, op=OP.add)

        def emit_collate(G):
            # data_all[po, m, b*20+(ki,u)] = kernc[po+(b-2), kx=4-b, m, ki, u]
            # via PE shift matmuls (zero-padded at block edges by the matrices)
            pda = psc.tile([128, 512], F32, name=f'pda_{G}', tag='pcs')
            for b in range(5):
                rhs = _ap(kernc[:], (4 - b) * 320 + GM * G * 20, [[1600, 128], [1, 80]])
                nc.tensor.matmul(pda[:, b * 80:b * 80 + 80], sh_sb[:, b, :], rhs,
                                 start=True, stop=True)
            nc.scalar.activation(
                out=_ap(data_all[:], GM * G * 100, [[1600, 128], [100, GM], [20, 5], [1, 20]]),
                in_=_ap(pda[:], 0, [[512, 128], [20, GM], [80, 5], [1, 20]]),
                func=AF.Copy, scale=1.0)

        def emit_pairs(G):
            for m in range(GM * G, GM * G + GM):
                banded1 = band.tile([128, 1536], BF16, name=f'band1_{m}', tag='band1')
                banded2 = band.tile([128, 1024], BF16, name=f'band2_{m}', tag='band2')
                nc.gpsimd.local_scatter(out_ap=banded1[:], data_ap=data_all[:, m, :],
                                        idxs_ap=idx1_sb[:], channels=128, num_elems=1536, num_idxs=100)
                nc.gpsimd.local_scatter(out_ap=banded2[:], data_ap=data_all[:, m, :],
                                        idxs_ap=idx2_sb[:], channels=128, num_elems=1024, num_idxs=100)
                for ch in range(2):
                    pcs = psc.tile([128, 512], F32, name=f'pcs_{m}_{ch}', tag='pcs')
                    for ki in range(5):
                        lhsT = _ap(xT2_sb[:], (m + ki) * 256 + ch * 128, [[20 * 256, 128], [1, 128]])
                        rhs = banded1[:, ki * 512:ki * 512 + 512] if ki < 3 \
                            else banded2[:, (ki - 3) * 512:(ki - 3) * 512 + 512]
                        nc.tensor.matmul(pcs[:], lhsT, rhs, start=(ki == 0), stop=(ki == 4))
                    rb = rowp.tile([128, 512], F32, name=f'rb_{m}_{ch}', tag='rb')
                    # alternate psum evac between ScalarE and (back-half idle) DVE
                    if (m + ch) % 2 == 0:
                        nc.scalar.activation(out=rb[:], in_=pcs[:], func=AF.Copy, scale=1.0)
                    else:
                        nc.vector.tensor_copy(out=rb[:], in_=pcs[:])
                    nc.sync.dma_start(
                        out=_ap(outp[:], ch * 128 * 8192 + 4 * m * 128,
                                [[8192, 128], [128, 4], [1, 128]]),
                        in_=rb[:])

        # interleaved emission: per-engine queues are in-order, so collates
        # (PE, dependent on DVE asm) are placed after the conv groups they
        # must not delay; scatters start right after asm(G0)+collate(G0)
        emit_conv(0, 0); emit_conv(0, 1); emit_exp_tr(0); emit_softmax(0)
        emit_conv(1, 0); emit_conv(1, 1); emit_exp_tr(1); emit_softmax(1)
        emit_off_tr(0); emit_shift(0); emit_shift(1); emit_w9(0); emit_asm(0)
        emit_conv(2, 0); emit_conv(2, 1); emit_exp_tr(2); emit_softmax(2)
        emit_collate(0); emit_shift(2)
        emit_conv(3, 0); emit_conv(3, 1); emit_exp_tr(3); emit_off_tr(1)
        emit_softmax(3); emit_shift(3); emit_w9(1); emit_asm(1); emit_collate(1)
        emit_pairs(0); emit_asm(2); emit_collate(2)
        emit_pairs(1); emit_asm(3); emit_collate(3)
        emit_pairs(2)
        emit_pairs(3)
    nc.finalize()
    return nc


_PROGRAM = None
_SCAT = build_scatter_tables()


def _get_program():
    global _PROGRAM
    if _PROGRAM is None:
        _PROGRAM = build_program()
    return _PROGRAM


def _prep_core_inputs(inputs, n, s):
    bf = np.float16
    x = np.asarray(inputs['x'][n], np.float32)
    h0 = 32 * s
    xw = np.zeros((C, 38, W), np.float32)
    for i, g in enumerate(range(h0 - 3, h0 + 35)):
        if 0 <= g < H:
            xw[:, i] = x[:, g]
    xwin = np.ascontiguousarray(xw.reshape(2, 128, 38 * 64)).astype(bf)
    xT2 = np.zeros((128, 20, C), np.float32)
    for jh in range(2):
        base = h0 + 16 * jh - 2
        for i in range(20):
            g = base + i
            if 0 <= g < H:
                xT2[64 * jh:64 * jh + 64, i] = x[:, g].T
    xT2 = np.ascontiguousarray(xT2.reshape(128, 20 * 256)).astype(bf)
    w_comp = np.asarray(inputs['w_comp'], np.float32)[:, :, 0, 0]
    wc = np.zeros((2, 128, 64), np.float32)
    for cg in range(2):
        wc[cg] = w_comp[:, cg * 128:(cg + 1) * 128].T
    wc = np.ascontiguousarray(wc.transpose(1, 0, 2).reshape(128, 2 * 64)).astype(bf)
    w_ker = np.asarray(inputs['w_ker'], np.float32)
    w_off = np.asarray(inputs['w_off'], np.float32)
    # mask channels permuted to k = kx*5 + ky so the collate DMA reads a
    # contiguous 20-elem (ki, u) slice per horizontal tap kx
    kperm = np.array([ky * 5 + kx for kx in range(5) for ky in range(5)])
    wk = np.zeros((9, 64, 57), np.float32)
    for t in range(9):
        wk[t, :, 0:8] = w_off[:, :, t // 3, t % 3].T
        wk[t, :, 32:57] = w_ker[kperm, :, t // 3, t % 3].T
    wk = np.ascontiguousarray(wk.transpose(1, 0, 2).reshape(64, 9 * 57)).astype(bf)
    bcov = np.zeros((57, 1), np.float32)
    bcov[0:8, 0] = np.asarray(inputs['b_off'], np.float32)
    bcov[32:57, 0] = np.asarray(inputs['b_ker'], np.float32)[kperm]
    idx1, idx2 = _SCAT
    wv = np.tile(np.arange(64, dtype=np.float32), 2).reshape(128, 1)
    hr = (h0 + 16 * (np.arange(128)[:, None] // 64)
          + np.arange(16, dtype=np.float32)[None, :]).astype(np.float32)
    # shift matrices: shmat[pc, b*128+po] = 1 iff pc == po + (b-2), same 64-block
    sh = np.zeros((128, 5, 128), np.float16)
    for b in range(5):
        d = b - 2
        for po in range(128):
            pc = po + d
            if 0 <= pc < 128 and pc // 64 == po // 64:
                sh[pc, b, po] = 1.0
    return {
        'xwin': xwin, 'xT2': xT2, 'wc': wc, 'wk': wk, 'bco': bcov,
        'bcomp': np.asarray(inputs['b_comp'], np.float32).reshape(64, 1),
        'wvec': wv,
        'w63': (63.0 - wv).astype(np.float32),
        'hrow': np.ascontiguousarray(hr),
        'y63': np.ascontiguousarray(63.0 - hr),
        'ident': np.eye(128, dtype=np.float32),
        'shmat': np.ascontiguousarray(sh.reshape(128, 5 * 128)),
        'idx1': idx1, 'idx2': idx2,
    }


def kernel(**inputs):
    nc = _get_program()
    core_ids = list(range(8))
    in_maps = [_prep_core_inputs(inputs, cid // 2, cid % 2) for cid in core_ids]
    res = run_bass_kernel_spmd(nc, in_maps, core_ids)
    out = np.zeros((N, C, 128, 128), np.float32)
    for cid in core_ids:
        n, s = cid // 2, cid % 2
        op = np.asarray(res.results[cid]['outp']).reshape(256, 64, 128)
        out[n, :, s::2] = op
    return out


if __name__ == '__main__':
    d = np.load('/root/problem/ref_io.npz')
    inp = {k: d[k] for k in ('x', 'w_comp', 'b_comp', 'w_ker', 'b_ker', 'w_off', 'b_off')}
    out = kernel(**inp)
    ref = d['out']
    err = np.abs(out - ref).max()
    print('max abs err:', err, 'rel:', err / np.abs(ref).max())


# revision 27
# speedup vs baseline: 1.6845x; 1.0055x over previous
"""Trainium2 Bass kernel for nn_DLUPack (CARAFE-style dynamic upsampling).

Sharding: 8 cores = (batch n in [0,4)) x (output-row-parity s in {0,1});
core (n, s) computes low-res rows hh in [32s, 32s+32) -> all parity-s output rows.

Reference output mapping (its reshape scrambles positions):
  ref[n, c, 2y+i, 2x+j] = sum_k patches[c, hh, ww, k] * kern[hh, ww, k, u]
  with hh = 32s + 16jh + m:  row r = 8m + 2(ww//16) + s, col = 8*(ww%16) + 2u + jh.

Device pipeline per core (all post-conv tensors packed [128 = (jh, w)], FD halved
vs the [64, 2*FD] layout so DVE ops run in half the cycles):
  1. compressor 1x1 conv (PE, fp16) -> cx [64, 38, 66]
  2. offset+mask 3x3 convs (9 accumulated MMs) -> psum [57, .]: off ch 0-7, mask 32-56
     (mask channels host-permuted to k = kx*5 + ky order)
  3. exp in ACT evac; 20 PE-transposes of row-pairs (r, r+16) -> expT [128,20,25] f32
     softmax denom via free-dim reduce; expT4 = expT*recT bcast-u -> fp16 [128,20,100]
     partition-shifted copies expT4_{p1,m1} via SBUF-SBUF DMA (edges zeroed from zed)
  4. offset PE-transpose (row-pairs) -> deltT [128,16,8]; indicator bilinear W9 (DVE)
  5. kernc assembly [128, 16m, 25k, 4u]: 9 bcast-mult (1x) + 8 dense adds (2x)
  6. collate: 5 partition-shifted strided DMAs kernc -> data_all [128, 16, 100]
     (contaminated edge slots are skipped by the scatter idx tables = -1)
  7. per pair m: 2 local_scatter (GPSIMD) -> banded [128, 3x512 | 2x512]
  8. carafe: 5 accumulated MMs [128,128]x[128,512] per (pair, c-half) -> psum [128,512]
  9. ACT evac -> DMA out 4 contiguous output rows
"""
import sys
import numpy as np

sys.path.insert(0, '/opt/trn_rl_repo')

import ml_dtypes  # noqa: E402
from contextlib import ExitStack  # noqa: E402

import concourse.bass as bass  # noqa: E402
import concourse.tile as tile  # noqa: E402
from concourse import mybir, bacc  # noqa: E402
from concourse.bass_utils import run_bass_kernel_spmd  # noqa: E402

F32 = mybir.dt.float32
BF16 = mybir.dt.float16  # NOTE: fp16 (better mantissa), name kept for brevity
I16 = mybir.dt.int16
AF = mybir.ActivationFunctionType
OP = mybir.AluOpType

N, C, H, W = 4, 256, 64, 64


def _ap(base, off_elems, dims):
    return bass.AP(tensor=base.tensor, offset=base.offset + off_elems, ap=[list(d) for d in dims])


def build_scatter_tables():
    idx1 = -np.ones((128, 100), np.int16)
    idx2 = -np.ones((128, 100), np.int16)
    for p in range(128):
        jh, wpp = p // 64, p % 64
        for b in range(5):
            w = wpp + b - 2
            if not (0 <= w < 64):
                continue
            q, wl = w // 16, w % 16
            for ki in range(5):
                for u in range(4):
                    col = q * 128 + 8 * wl + 2 * u + jh
                    qidx = (b * 5 + ki) * 4 + u
                    if ki < 3:
                        idx1[p, qidx] = ki * 512 + col
                    else:
                        idx2[p, qidx] = (ki - 3) * 512 + col
    return idx1, idx2


def build_program():
    nc = bacc.Bacc(None, target_bir_lowering=False, debug=True)

    xwin = nc.declare_dram_parameter('xwin', [2, 128, 38 * 64], BF16, isOutput=False)
    xT2 = nc.declare_dram_parameter('xT2', [128, 20 * 256], BF16, isOutput=False)
    wc = nc.declare_dram_parameter('wc', [128, 2 * 64], BF16, isOutput=False)
    wk = nc.declare_dram_parameter('wk', [64, 9 * 57], BF16, isOutput=False)
    bco = nc.declare_dram_parameter('bco', [57, 1], F32, isOutput=False)
    bcomp = nc.declare_dram_parameter('bcomp', [64, 1], F32, isOutput=False)
    wvec = nc.declare_dram_parameter('wvec', [128, 1], F32, isOutput=False)
    w63 = nc.declare_dram_parameter('w63', [128, 1], F32, isOutput=False)
    hrow = nc.declare_dram_parameter('hrow', [128, 16], F32, isOutput=False)
    y63 = nc.declare_dram_parameter('y63', [128, 16], F32, isOutput=False)
    ident = nc.declare_dram_parameter('ident', [128, 128], F32, isOutput=False)
    shmat = nc.declare_dram_parameter('shmat', [128, 5 * 128], BF16, isOutput=False)
    idx1 = nc.declare_dram_parameter('idx1', [128, 100], I16, isOutput=False)
    idx2 = nc.declare_dram_parameter('idx2', [128, 100], I16, isOutput=False)
    outp = nc.declare_dram_parameter('outp', [256, 64 * 128], F32, isOutput=True)

    with tile.TileContext(nc) as tc, ExitStack() as ctx:
        sing = ctx.enter_context(tc.tile_pool(name='sing', bufs=1))
        work = ctx.enter_context(tc.tile_pool(name='work', bufs=1))
        band = ctx.enter_context(tc.tile_pool(name='band', bufs=6))
        rowp = ctx.enter_context(tc.tile_pool(name='rowp', bufs=4))
        psum = ctx.enter_context(tc.psum_pool(name='ps', bufs=3))
        psc = ctx.enter_context(tc.psum_pool(name='psc', bufs=3))
        ptr = ctx.enter_context(tc.psum_pool(name='ptr', bufs=1))

        def load(shape, dtype, src, eng=None):
            t = sing.tile(shape, dtype, name=f'ld_{src.tensor.name if hasattr(src, "tensor") else id(src)}')
            (eng or nc.sync).dma_start(out=t[:], in_=src[:])
            return t

        # small params first on sync queue so compute can start ASAP;
        # ident early (warm-up matmuls need it)
        wc_sb = load([128, 2, 64], BF16, wc)
        bcomp_sb = load([64, 1], F32, bcomp)
        id_sb = load([128, 128], F32, ident)
        # xwin loaded in row chunks alternating queues so compressor group g
        # starts after ~1/5 of the load
        xwin_sb = sing.tile([128, 2, 38 * 64], BF16)
        for ck in range(5):
            r0 = ck * 8
            rows = min(8, 38 - r0)
            for cg in range(2):
                eng = nc.sync if cg == 0 else nc.scalar
                eng.dma_start(
                    out=xwin_sb[:, cg, r0 * 64:(r0 + rows) * 64],
                    in_=_ap(xwin[:], cg * 128 * 2432 + r0 * 64,
                            [[2432, 128], [1, rows * 64]]))
        wk_sb = load([64, 9, 57], BF16, wk, eng=nc.gpsimd)
        bco_sb = load([57, 1], F32, bco, eng=nc.gpsimd)
        wvec_sb = load([128, 1], F32, wvec, eng=nc.gpsimd)
        w63_sb = load([128, 1], F32, w63, eng=nc.gpsimd)
        hrow_sb = load([128, 16], F32, hrow, eng=nc.gpsimd)
        y63_sb = load([128, 16], F32, y63, eng=nc.gpsimd)
        sh_sb = load([128, 5, 128], BF16, shmat, eng=nc.gpsimd)
        xT2_sb = load([128, 20 * 256], BF16, xT2, eng=nc.scalar)
        idx1_sb = load([128, 100], I16, idx1, eng=nc.scalar)
        idx2_sb = load([128, 100], I16, idx2, eng=nc.scalar)

        # PE warm-up: keep TensorE busy during input-DMA wait so HAM reaches 8/8
        pw = psc.tile([128, 512], F32, name='pcs_warm', tag='pcs')
        for _ in range(48):
            nc.tensor.matmul(pw[0:64, 0:64], id_sb[:, 0:64], id_sb[:, 0:64], start=True, stop=True)

        # ---- 1. compressor ----
        cx_sb = work.tile([64, 38, 66], BF16)
        nc.vector.memset(_ap(cx_sb[:], 0, [[38 * 66, 64], [66, 38], [1, 1]]), 0.0)
        nc.vector.memset(_ap(cx_sb[:], 65, [[38 * 66, 64], [66, 38], [1, 1]]), 0.0)
        for grp in range(5):
            g0 = grp * 8
            rows = min(8, 38 - g0)
            nn = rows * 64
            pcs = psum.tile([64, 512], F32)
            for cg in range(2):
                nc.tensor.matmul(pcs[:, :nn], wc_sb[:, cg, :],
                                 xwin_sb[:, cg, g0 * 64:g0 * 64 + nn],
                                 start=(cg == 0), stop=(cg == 1))
            nc.scalar.activation(
                out=_ap(cx_sb[:], g0 * 66 + 1, [[38 * 66, 64], [66, rows], [1, 64]]),
                in_=_ap(pcs[:], 0, [[512, 64], [64, rows], [1, 64]]),
                func=AF.Identity, bias=bcomp_sb[:], scale=1.0)

        # ---- 2-4. offset+mask convs, transposes, softmax, W9 -- emitted as an
        # interleaved pipeline so asm(G0)/scatters start right after conv grp1.
        # expS pair-interleaved (i, jh, w); conv row r = 16jh + i covers h = r-2
        expS = work.tile([25, 20, 2, 64], F32)
        offS = work.tile([8, 16, 2, 64], F32)
        expT = work.tile([128, 20, 25], F32)
        sumT = work.tile([128, 20], F32)
        recT = work.tile([128, 20], F32)
        expT4 = work.tile([128, 20, 25, 4], BF16)
        expT4_p1 = work.tile([128, 20, 25, 4], BF16)   # [p] = expT4[p+1]
        expT4_m1 = work.tile([128, 20, 25, 4], BF16)   # [p] = expT4[p-1]
        deltT = work.tile([128, 16, 8], BF16)
        pt = ptr.tile([128, 512], F32, name='pt_exp')
        po_t = ptr.tile([128, 512], F32, name='po_w')

        def emit_conv(grp, jh):
            i0 = grp * 5
            nn = 5 * 64
            pcs = psum.tile([57, 320], F32)
            for t in range(9):
                dy, dx = t // 3, t % 3
                rhs = _ap(cx_sb[:], (16 * jh + i0 + dy) * 66 + dx,
                          [[38 * 66, 64], [66, 5], [1, 64]])
                nc.tensor.matmul(pcs[:, :nn], wk_sb[:, t, :], rhs,
                                 start=(t == 0), stop=(t == 8))
            nc.scalar.activation(
                out=_ap(expS[:], (i0 * 2 + jh) * 64, [[20 * 128, 25], [128, 5], [1, 64]]),
                in_=_ap(pcs[:], 32 * 320, [[320, 25], [64, 5], [1, 64]]),
                func=AF.Exp, bias=bco_sb[32:57], scale=1.0)
            lo, hi = max(i0, 2), min(i0 + 5, 18)
            if lo < hi:
                # bias-add evac on ScalarE to keep the DVE queue clear for asm
                nc.scalar.activation(
                    out=_ap(offS[:], ((lo - 2) * 2 + jh) * 64,
                            [[16 * 128, 8], [128, hi - lo], [1, 64]]),
                    in_=_ap(pcs[:], (lo - i0) * 64, [[320, 8], [64, hi - lo], [1, 64]]),
                    func=AF.Identity, bias=bco_sb[0:8], scale=1.0)

        def emit_exp_tr(grp):
            for i in range(grp * 5, grp * 5 + 5):
                nc.tensor.transpose(pt[:, i * 25:i * 25 + 25],
                                    _ap(expS[:], i * 128, [[20 * 128, 25], [1, 128]]),
                                    id_sb[0:25, 0:25])
            nc.scalar.activation(
                out=_ap(expT[:], grp * 125, [[500, 128], [1, 125]]),
                in_=_ap(pt[:], grp * 125, [[512, 128], [1, 125]]),
                func=AF.Copy, scale=1.0)

        def emit_softmax(grp):
            g5 = grp * 5
            nc.vector.tensor_reduce(out=sumT[:, g5:g5 + 5], in_=expT[:, g5:g5 + 5, :],
                                    axis=mybir.AxisListType.X, op=OP.add)
            nc.vector.reciprocal(out=recT[:, g5:g5 + 5], in_=sumT[:, g5:g5 + 5])
            nc.vector.tensor_tensor(
                out=_ap(expT4[:], g5 * 100, [[2000, 128], [100, 5], [4, 25], [1, 4]]),
                in0=_ap(expT[:], g5 * 25, [[500, 128], [25, 5], [1, 25], [0, 4]]),
                in1=_ap(recT[:], g5, [[20, 128], [1, 5], [0, 25], [0, 4]]), op=OP.mult)

        def emit_shift(grp):
            # partition-shifted variants (w +- 1) via PE shift matmuls; shift
            # matrices zero block edges (SBUF-SBUF DMA shifts are ~13GB/s, avoid)
            for dst, sidx in ((expT4_p1, 3), (expT4_m1, 1)):
                psh = psc.tile([128, 512], F32, name=f'psh_{sidx}_{grp}', tag='pcs')
                nc.tensor.matmul(psh[:, 0:500], sh_sb[:, sidx, :],
                                 _ap(expT4[:], grp * 500, [[2000, 128], [1, 500]]),
                                 start=True, stop=True)
                nc.scalar.activation(
                    out=_ap(dst[:], grp * 500, [[2000, 128], [1, 500]]),
                    in_=_ap(psh[:], 0, [[512, 128], [1, 500]]),
                    func=AF.Copy, scale=1.0)

        def emit_off_tr(half):
            for m in range(half * 8, half * 8 + 8):
                nc.tensor.transpose(po_t[:, m * 8:m * 8 + 8],
                                    _ap(offS[:], m * 128, [[16 * 128, 8], [1, 128]]),
                                    id_sb[0:8, 0:8])
            nc.scalar.activation(
                out=_ap(deltT[:], half * 64, [[128, 128], [1, 64]]),
                in_=_ap(po_t[:], half * 64, [[512, 128], [1, 64]]),
                func=AF.Copy, scale=1.0)

        def wt(nm):
            return work.tile([128, 64], BF16, name=nm)

        t1, t2 = wt('t1'), wt('t2')
        gxc, x0r, wxt, omwx, x1r = wt('gxc'), wt('x0r'), wt('wxt'), wt('omwx'), wt('x1r')
        gyc, y0r, wyt, omwy, y1r = wt('gyc'), wt('y0r'), wt('wyt'), wt('omwy'), wt('y1r')
        ia, ib = wt('ia'), wt('ib')
        cwx = work.tile([128, 3, 64], BF16)
        rwy = work.tile([128, 3, 64], BF16)
        # W9 broadcast over ki: [t, m(16), ki(5), u(4)] so asm APs stay 3-free-dim
        W9b = work.tile([128, 9 * 320], BF16)

        def emit_w9(half):
            # whole indicator/bilinear chain on the m-slice [8h..8h+8)
            s = half * 32

            def sl(t):
                return _ap(t[:], s, [[64, 128], [1, 32]])

            def dview(chbase):
                return _ap(deltT[:], chbase + half * 64, [[128, 128], [8, 8], [1, 4]])

            def r4(t):
                return _ap(t[:], s, [[64, 128], [4, 8], [1, 4]])

            hrow_bc = _ap(hrow_sb[:], half * 8, [[16, 128], [1, 8], [0, 4]])
            y63_bc = _ap(y63_sb[:], half * 8, [[16, 128], [1, 8], [0, 4]])
            ts, tt = nc.vector.tensor_scalar, nc.vector.tensor_tensor
            ts(out=sl(t1), in0=dview(0), scalar1=wvec_sb[:], scalar2=None, op0=OP.add)
            ts(out=sl(t2), in0=sl(t1), scalar1=0.0, scalar2=63.0, op0=OP.max, op1=OP.min)
            ts(out=sl(gxc), in0=sl(t2), scalar1=wvec_sb[:], scalar2=None, op0=OP.subtract)
            ts(out=sl(x0r), in0=sl(gxc), scalar1=0.0, scalar2=-1.0, op0=OP.is_lt, op1=OP.mult)
            tt(out=sl(wxt), in0=sl(gxc), in1=sl(x0r), op=OP.subtract)
            ts(out=sl(omwx), in0=sl(wxt), scalar1=-1.0, scalar2=1.0, op0=OP.mult, op1=OP.add)
            ts(out=sl(x1r), in0=sl(x0r), scalar1=1.0, scalar2=w63_sb[:], op0=OP.add, op1=OP.min)

            tt(out=r4(t1), in0=dview(4), in1=hrow_bc, op=OP.add)
            ts(out=sl(t2), in0=sl(t1), scalar1=0.0, scalar2=63.0, op0=OP.max, op1=OP.min)
            tt(out=r4(gyc), in0=r4(t2), in1=hrow_bc, op=OP.subtract)
            ts(out=sl(y0r), in0=sl(gyc), scalar1=0.0, scalar2=-1.0, op0=OP.is_lt, op1=OP.mult)
            tt(out=sl(wyt), in0=sl(gyc), in1=sl(y0r), op=OP.subtract)
            ts(out=sl(omwy), in0=sl(wyt), scalar1=-1.0, scalar2=1.0, op0=OP.mult, op1=OP.add)
            ts(out=sl(t1), in0=sl(y0r), scalar1=1.0, scalar2=None, op0=OP.add)
            tt(out=r4(y1r), in0=r4(t1), in1=y63_bc, op=OP.min)

            for i, e in enumerate((-1.0, 0.0, 1.0)):
                ts(out=sl(ia), in0=sl(x0r), scalar1=e, scalar2=None, op0=OP.is_equal)
                ts(out=sl(ib), in0=sl(x1r), scalar1=e, scalar2=None, op0=OP.is_equal)
                tt(out=sl(ia), in0=sl(ia), in1=sl(omwx), op=OP.mult)
                tt(out=sl(ib), in0=sl(ib), in1=sl(wxt), op=OP.mult)
                tt(out=_ap(cwx[:], i * 64 + s, [[192, 128], [1, 32]]),
                   in0=sl(ia), in1=sl(ib), op=OP.add)
                ts(out=sl(ia), in0=sl(y0r), scalar1=e, scalar2=None, op0=OP.is_equal)
                ts(out=sl(ib), in0=sl(y1r), scalar1=e, scalar2=None, op0=OP.is_equal)
                tt(out=sl(ia), in0=sl(ia), in1=sl(omwy), op=OP.mult)
                tt(out=sl(ib), in0=sl(ib), in1=sl(wyt), op=OP.mult)
                tt(out=_ap(rwy[:], i * 64 + s, [[192, 128], [1, 32]]),
                   in0=sl(ia), in1=sl(ib), op=OP.add)
            for iy in range(3):
                for ix in range(3):
                    nc.vector.tensor_tensor(
                        out=_ap(W9b[:], (iy * 3 + ix) * 320 + half * 160,
                                [[9 * 320, 128], [20, 8], [4, 5], [1, 4]]),
                        in0=_ap(rwy[:], iy * 64 + s, [[192, 128], [4, 8], [0, 5], [1, 4]]),
                        in1=_ap(cwx[:], ix * 64 + s, [[192, 128], [4, 8], [0, 5], [1, 4]]),
                        op=OP.mult)

        # ---- 5-9 software-pipelined by m-groups ----
        # kernc layout [p, kx(5), m(16), ki(5), u(4)] so per-(kx, m-group) slices
        # are contiguous for the collate shift-matmul rhs
        kernc = work.tile([128, 5, 16, 5, 4], BF16)
        tmp = work.tile([128, 400], BF16)
        data_all = work.tile([128, 16, 100], BF16)
        exp_by_ex = {-1: expT4_m1, 0: expT4, 1: expT4_p1}
        NG = 4
        GM = 16 // NG

        def emit_asm(G):
            kv = _ap(kernc[:], GM * G * 20, [[1600, 128], [320, 5], [20, GM], [1, 20]])
            tv = _ap(tmp[:], 0, [[400, 128], [80, 5], [20, GM], [1, 20]])
            first = True
            for iy, ey in enumerate((-1, 0, 1)):
                for ix, ex in enumerate((-1, 0, 1)):
                    # expT4 [row(20), kx(5), ki(5), u(4)] -> read as [kx, m, kiu]
                    mv = _ap(exp_by_ex[ex][:], (2 + ey + GM * G) * 100,
                             [[2000, 128], [20, 5], [100, GM], [1, 20]])
                    wv = _ap(W9b[:], (iy * 3 + ix) * 320 + GM * G * 20,
                             [[9 * 320, 128], [0, 5], [20, GM], [1, 20]])
                    if first:
                        nc.vector.tensor_tensor(out=kv, in0=wv, in1=mv, op=OP.mult)
                        first = False
                    else:
                        nc.vector.tensor_tensor(out=tv, in0=wv, in1=mv, op=OP.mult)
                        nc.vector.tensor_tensor(out=kv, in0=kv, in1=tv, op=OP.add)

        def emit_collate(G):
            # data_all[po, m, b*20+(ki,u)] = kernc[po+(b-2), kx=4-b, m, ki, u]
            # via PE shift matmuls (zero-padded at block edges by the matrices)
            pda = psc.tile([128, 512], F32, name=f'pda_{G}', tag='pcs')
            for b in range(5):
                rhs = _ap(kernc[:], (4 - b) * 320 + GM * G * 20, [[1600, 128], [1, 80]])
                nc.tensor.matmul(pda[:, b * 80:b * 80 + 80], sh_sb[:, b, :], rhs,
                                 start=True, stop=True)
            nc.scalar.activation(
                out=_ap(data_all[:], GM * G * 100, [[1600, 128], [100, GM], [20, 5], [1, 20]]),
                in_=_ap(pda[:], 0, [[512, 128], [20, GM], [80, 5], [1, 20]]),
                func=AF.Copy, scale=1.0)

        def emit_pairs(G):
            for m in range(GM * G, GM * G + GM):
                banded1 = band.tile([128, 1536], BF16, name=f'band1_{m}', tag='band1')
                banded2 = band.tile([128, 1024], BF16, name=f'band2_{m}', tag='band2')
                nc.gpsimd.local_scatter(out_ap=banded1[:], data_ap=data_all[:, m, :],
                                        idxs_ap=idx1_sb[:], channels=128, num_elems=1536, num_idxs=100)
                nc.gpsimd.local_scatter(out_ap=banded2[:], data_ap=data_all[:, m, :],
                                        idxs_ap=idx2_sb[:], channels=128, num_elems=1024, num_idxs=100)
                for ch in range(2):
                    pcs = psc.tile([128, 512], F32, name=f'pcs_{m}_{ch}', tag='pcs')
                    for ki in range(5):
                        lhsT = _ap(xT2_sb[:], (m + ki) * 256 + ch * 128, [[20 * 256, 128], [1, 128]])
                        rhs = banded1[:, ki * 512:ki * 512 + 512] if ki < 3 \
                            else banded2[:, (ki - 3) * 512:(ki - 3) * 512 + 512]
                        nc.tensor.matmul(pcs[:], lhsT, rhs, start=(ki == 0), stop=(ki == 4))
                    rb = rowp.tile([128, 512], F32, name=f'rb_{m}_{ch}', tag='rb')
                    # alternate psum evac between ScalarE and (back-half idle) DVE
                    if (m + ch) % 2 == 0:
                        nc.scalar.activation(out=rb[:], in_=pcs[:], func=AF.Copy, scale=1.0)
                    else:
                        nc.vector.tensor_copy(out=rb[:], in_=pcs[:])
                    nc.sync.dma_start(
                        out=_ap(outp[:], ch * 128 * 8192 + 4 * m * 128,
                                [[8192, 128], [128, 4], [1, 128]]),
                        in_=rb[:])

        # interleaved emission: per-engine queues are in-order, so collates
        # (PE, dependent on DVE asm) are placed after the conv groups they
        # must not delay; scatters start right after asm(G0)+collate(G0)
        emit_conv(0, 0); emit_conv(0, 1); emit_exp_tr(0); emit_softmax(0)
        emit_conv(1, 0); emit_conv(1, 1); emit_exp_tr(1); emit_softmax(1)
        emit_off_tr(0); emit_shift(0); emit_shift(1); emit_w9(0); emit_asm(0)
        emit_conv(2, 0); emit_conv(2, 1); emit_exp_tr(2); emit_softmax(2)
        emit_collate(0); emit_shift(2)
        emit_conv(3, 0); emit_conv(3, 1); emit_exp_tr(3); emit_off_tr(1)
        emit_softmax(3); emit_shift(3); emit_w9(1); emit_asm(1); emit_collate(1)
        emit_pairs(0); emit_asm(2); emit_collate(2)
        emit_pairs(1); emit_asm(3); emit_collate(3)
        emit_pairs(2)
        emit_pairs(3)
    nc.finalize()
    return nc


_PROGRAM = None
_SCAT = build_scatter_tables()


def _get_program():
    global _PROGRAM
    if _PROGRAM is None:
        _PROGRAM = build_program()
    return _PROGRAM


def _prep_core_inputs(inputs, n, s):
    bf = np.float16
    x = np.asarray(inputs['x'][n], np.float32)
    h0 = 32 * s
    xw = np.zeros((C, 38, W), np.float32)
    for i, g in enumerate(range(h0 - 3, h0 + 35)):
        if 0 <= g < H:
            xw[:, i] = x[:, g]
    xwin = np.ascontiguousarray(xw.reshape(2, 128, 38 * 64)).astype(bf)
    xT2 = np.zeros((128, 20, C), np.float32)
    for jh in range(2):
        base = h0 + 16 * jh - 2
        for i in range(20):
            g = base + i
            if 0 <= g < H:
                xT2[64 * jh:64 * jh + 64, i] = x[:, g].T
    xT2 = np.ascontiguousarray(xT2.reshape(128, 20 * 256)).astype(bf)
    w_comp = np.asarray(inputs['w_comp'], np.float32)[:, :, 0, 0]
    wc = np.zeros((2, 128, 64), np.float32)
    for cg in range(2):
        wc[cg] = w_comp[:, cg * 128:(cg + 1) * 128].T
    wc = np.ascontiguousarray(wc.transpose(1, 0, 2).reshape(128, 2 * 64)).astype(bf)
    w_ker = np.asarray(inputs['w_ker'], np.float32)
    w_off = np.asarray(inputs['w_off'], np.float32)
    # mask channels permuted to k = kx*5 + ky so the collate DMA reads a
    # contiguous 20-elem (ki, u) slice per horizontal tap kx
    kperm = np.array([ky * 5 + kx for kx in range(5) for ky in range(5)])
    wk = np.zeros((9, 64, 57), np.float32)
    for t in range(9):
        wk[t, :, 0:8] = w_off[:, :, t // 3, t % 3].T
        wk[t, :, 32:57] = w_ker[kperm, :, t // 3, t % 3].T
    wk = np.ascontiguousarray(wk.transpose(1, 0, 2).reshape(64, 9 * 57)).astype(bf)
    bcov = np.zeros((57, 1), np.float32)
    bcov[0:8, 0] = np.asarray(inputs['b_off'], np.float32)
    bcov[32:57, 0] = np.asarray(inputs['b_ker'], np.float32)[kperm]
    idx1, idx2 = _SCAT
    wv = np.tile(np.arange(64, dtype=np.float32), 2).reshape(128, 1)
    hr = (h0 + 16 * (np.arange(128)[:, None] // 64)
          + np.arange(16, dtype=np.float32)[None, :]).astype(np.float32)
    # shift matrices: shmat[pc, b*128+po] = 1 iff pc == po + (b-2), same 64-block
    sh = np.zeros((128, 5, 128), np.float16)
    for b in range(5):
        d = b - 2
        for po in range(128):
            pc = po + d
            if 0 <= pc < 128 and pc // 64 == po // 64:
                sh[pc, b, po] = 1.0
    return {
        'xwin': xwin, 'xT2': xT2, 'wc': wc, 'wk': wk, 'bco': bcov,
        'bcomp': np.asarray(inputs['b_comp'], np.float32).reshape(64, 1),
        'wvec': wv,
        'w63': (63.0 - wv).astype(np.float32),
        'hrow': np.ascontiguousarray(hr),
        'y63': np.ascontiguousarray(63.0 - hr),
        'ident': np.eye(128, dtype=np.float32),
        'shmat': np.ascontiguousarray(sh.reshape(128, 5 * 128)),
        'idx1': idx1, 'idx2': idx2,
    }


def kernel(**inputs):
    nc = _get_program()
    core_ids = list(range(8))
    in_maps = [_prep_core_inputs(inputs, cid // 2, cid % 2) for cid in core_ids]
    res = run_bass_kernel_spmd(nc, in_maps, core_ids)
    out = np.zeros((N, C, 128, 128), np.float32)
    for cid in core_ids:
        n, s = cid // 2, cid % 2
        op = np.asarray(res.results[cid]['outp']).reshape(256, 64, 128)
        out[n, :, s::2] = op
    return out


if __name__ == '__main__':
    d = np.load('/root/problem/ref_io.npz')
    inp = {k: d[k] for k in ('x', 'w_comp', 'b_comp', 'w_ker', 'b_ker', 'w_off', 'b_off')}
    out = kernel(**inp)
    ref = d['out']
    err = np.abs(out - ref).max()
    print('max abs err:', err, 'rel:', err / np.abs(ref).max())
